# revision 26
# baseline (speedup 1.0000x reference)
"""BiasAttention Trainium2 Bass kernel (v3).

Computes, for x:[B,Q,CV], bias1:[B,H,Q,Q], bias2:[B,1,Q,Q], W_v/W_g:[CV,H*CH],
W_o:[H*CH,CV]:

    v = (x @ W_v) viewed [B,Q,H,CH]
    a = softmax(bias1 + bias2, axis=-1)
    o = einsum('bhqk,bhkd->bhqd', a, v) * sigmoid(x @ W_g)
    return o @ W_o

Sharding: data-parallel over the query dim Q across 8 NeuronCores (each core
computes QL=256 query rows end-to-end; no collectives).

v3 design notes (per core):
  - All host inputs are pre-arranged so every DMA lands 128 partitions with
    >=4KB contiguous per-partition runs (line-rate HBM DMA):
      b1h[b,h,p,kb*QL+q] = bias1[b,h,q0+q,kb*128+p]   (16KB/partition rows)
      b2h[b,p,kb*QL+q]   = bias2[b,0,q0+q,kb*128+p]
      xth[cb,p,j]        = x[j,cb*128+p]  (x transposed on host)
  - Startup-critical tensors (x^T, W_v, b2, b1) load f32 over the two HWDGE
    queues (sync + scalar); the v matmul runs f32r x f32r via bitcast (full
    rate at 512 free dim).  Only the small gate-path weights (wg, xl^T, wo)
    cast to bf16 through gpsimd SWDGE.
  - s = exp(b1+b2) in bf16 (attention matmul bf16 x bf16).
  - ScalarE runs ONLY Exp / Tanh / Copy (one ACT table set, no reloads).
    sigmoid(y) = 0.5*(1+tanh(y/2)); the 0.5 folds into the row-sum trick by
    setting the appended ones-column of v_aug to 2.0 (rowsum_psum = 2*sum(s),
    recip = 0.5/sum), and (1+tanh)*recip is one DVE scalar_tensor_tensor.
  - Row-sum reciprocal avoids single-partition DVE ops (1.75us each): ACT
    copies the PSUM row to SBUF (0.5us), gpsimd partition-broadcasts it to 64
    partitions, DVE takes the reciprocal wide.
  - Work is spread: z-adds mostly DVE with 1-in-4 on gpsimd; v_aug casts
    alternate DVE/ACT; per-head epilogues are emitted one head late so the
    in-order DVE stream never blocks the next head's add.
"""

import contextlib

import numpy as np


def _ensure_concourse():
    try:
        import concourse  # noqa: F401
    except ImportError:
        import sys

        for p in ("/root/.axon_site/_ro/trn_rl_repo", "/opt/trn_rl_repo"):
            if p not in sys.path:
                sys.path.insert(0, p)


_ensure_concourse()

import concourse.bacc as bacc  # noqa: E402
import concourse.mybir as mybir  # noqa: E402
import concourse.tile as tile  # noqa: E402
from concourse import bass_utils  # noqa: E402

F32 = mybir.dt.float32
F32R = mybir.dt.float32r
BF16 = mybir.dt.bfloat16
AF = mybir.ActivationFunctionType
ALU = mybir.AluOpType

# Problem dims (nn_BiasAttention): hardcoded per the harness contract.
CFG = dict(B=2, Q=2048, CV=512, H=8, CH=64, NCORES=8)


def build(cfg=None, repeat=1, ablate=()):
    c = dict(CFG if cfg is None else cfg)
    B, Q, CV, H, CH, NCORES = c["B"], c["Q"], c["CV"], c["H"], c["CH"], c["NCORES"]
    HD = H * CH
    QL = Q // NCORES  # query rows per core
    KB = Q // 128  # key blocks
    KQ = KB * QL  # free size of one (b,h) bias tile
    CVB = CV // 128
    DH1 = CH + 1  # head dim + ones column (row-sum trick)
    JC = 512  # x^T columns per staged chunk
    KPC = JC // 128  # key blocks covered per x^T chunk
    assert QL == 256 and CH == 64 and KQ == 4096

    nc = bacc.Bacc("TRN2", target_bir_lowering=False, debug=False, num_devices=NCORES)

    b1_d = nc.dram_tensor("b1h", [B, H, 128, KQ], F32, kind="ExternalInput")
    b2_d = nc.dram_tensor("b2h", [B, 128, KQ], F32, kind="ExternalInput")
    xt_d = nc.dram_tensor("xth", [CVB, 128, B * Q], F32, kind="ExternalInput")
    xlt_d = nc.dram_tensor("xlth", [CVB, 128, B * QL], F32, kind="ExternalInput")
    wv_d = nc.dram_tensor("wvh", [CVB, 128, HD], F32, kind="ExternalInput")
    wg_d = nc.dram_tensor("wgh", [CVB, 128, HD], F32, kind="ExternalInput")
    wo_d = nc.dram_tensor("woh", [H, CH, CV], F32, kind="ExternalInput")
    out_d = nc.dram_tensor("out", [B, QL, CV], F32, kind="ExternalOutput")

    with tile.TileContext(nc) as tc:
        loop = tc.For_i(0, repeat, 1) if repeat > 1 else contextlib.nullcontext()
        with loop, contextlib.ExitStack() as ctx:
            persist = ctx.enter_context(tc.tile_pool(name="persist", bufs=1))
            b1p = ctx.enter_context(tc.tile_pool(name="b1p", bufs=2))
            sp = ctx.enter_context(tc.tile_pool(name="sp", bufs=3))
            xp = ctx.enter_context(tc.tile_pool(name="xp", bufs=3))
            xbp = ctx.enter_context(tc.tile_pool(name="xbp", bufs=3))
            ep = ctx.enter_context(tc.tile_pool(name="ep", bufs=2))
            op = ctx.enter_context(tc.tile_pool(name="op", bufs=2))
            psA = ctx.enter_context(tc.tile_pool(name="psA", bufs=3, space="PSUM"))
            ps512 = ctx.enter_context(tc.tile_pool(name="ps512", bufs=2, space="PSUM"))
            psG = ctx.enter_context(tc.tile_pool(name="psG", bufs=1, space="PSUM"))
            psO = ctx.enter_context(tc.tile_pool(name="psO", bufs=1, space="PSUM"))

            # ---- persistent tiles ----
            b2_t = persist.tile([128, B * KQ], F32, name="b2_t", tag="b2_t")
            v_aug = [
                persist.tile([128, KB * H * DH1], BF16, name=f"vaug{b}", tag=f"vaug{b}")
                for b in range(B)
            ]
            xlt = persist.tile([128, CVB * B * QL], BF16, name="xlt", tag="xlt")
            wv_t = persist.tile([128, CVB * HD], BF16, name="wv_t", tag="wv_t")
            wg_t = persist.tile([128, CVB * HD], BF16, name="wg_t", tag="wg_t")
            wo_t = persist.tile([CH, H * CV], BF16, name="wo_t", tag="wo_t")
            gt_t = persist.tile([CH, H * QL], F32, name="gt_t", tag="gt_t")
            og_shared = persist.tile([CH, H * QL], BF16, name="og", tag="og")
            og_t = [og_shared for _ in range(B)]

            # ---- setup DMAs ----
            # PE warm-up source: memset a scratch strip of v_aug[0]; ~36
            # accumulating matmuls on it keep the HAM activity window busy so
            # the PE clock is at 2.4GHz when real v-staging matmuls arrive.
            nc.gpsimd.memset(v_aug[0][:, 0 : 2 * QL + DH1], 1.0)
            # b2[0] split across both HWDGE queues for startup latency
            nc.sync.dma_start(b2_t[:, 0 : KQ // 2], b2_d[0][:, 0 : KQ // 2])
            xt_tiles = {}

            xt_f32 = {}

            def stage_xt_dma(cidx, eng=None):
                """x^T chunk: f32 load (scalar HWDGE or gpsimd SWDGE)."""
                if cidx in xt_f32:
                    return
                eng = eng or nc.scalar
                xt = xp.tile([128, CVB * JC], F32, name=f"xt{cidx}", tag="xt")
                eng.dma_start(
                    xt[:].rearrange("p (cb j) -> p cb j", cb=CVB),
                    xt_d[:, :, cidx * JC : (cidx + 1) * JC].rearrange(
                        "cb p j -> p cb j"
                    ),
                )
                xt_f32[cidx] = xt

            def stage_xt_cast(cidx):
                """bf16 cast on DVE (even chunks) / ACT (odd chunks)."""
                if cidx in xt_tiles:
                    return
                stage_xt_dma(cidx)
                xtb = xbp.tile([128, CVB * JC], BF16, name=f"xtb{cidx}", tag="xtb")
                if cidx % 2 == 0:
                    nc.vector.tensor_copy(xtb[:], xt_f32[cidx][:])
                else:
                    nc.scalar.copy(xtb[:], xt_f32[cidx][:])
                xt_tiles[cidx] = xtb

            # wv staged f32 through an xt-pool slot (reused by later chunks);
            # it leads the scalar queue so it lands early
            wv_st = xp.tile([128, CVB * JC], F32, name="wv_st", tag="xt")
            nc.scalar.dma_start(
                wv_st[:, 0 : CVB * HD].rearrange("p (cb d) -> p cb d", cb=CVB),
                wv_d[:, :, :].rearrange("cb p d -> p cb d"),
            )
            nc.scalar.dma_start(
                b2_t[:, KQ // 2 : KQ], b2_d[0][:, KQ // 2 : KQ]
            )
            # b0's x^T chunks lead the gpsimd queue; gate-weight cast-DMAs
            # interleave behind the first two chunks
            stage_xt_dma(0, nc.gpsimd)
            stage_xt_dma(1, nc.gpsimd)
            nc.gpsimd.dma_start(
                wg_t[:].rearrange("p (cb d) -> p cb d", cb=CVB),
                wg_d[:, :, :].rearrange("cb p d -> p cb d"),
            )
            stage_xt_dma(2, nc.gpsimd)
            nc.gpsimd.dma_start(
                xlt[:].rearrange("p (cb j) -> p cb j", cb=CVB),
                xlt_d[:, :, :].rearrange("cb p j -> p cb j"),
            )
            stage_xt_dma(3, nc.gpsimd)
            # warm-up matmuls (results discarded; psA slot recycled later)
            warm_ps = psA.tile([DH1, 2 * QL], F32, name="warm", tag="oT")
            for i in range(36):
                nc.tensor.matmul(
                    warm_ps[:, 0:QL],
                    v_aug[0][:, 0:DH1],
                    v_aug[0][:, DH1 : DH1 + QL],
                    start=(i == 0),
                    stop=(i == 35),
                )
            # ones columns (value 2.0: folds the 0.5 of the tanh-sigmoid)
            for b in range(B):
                ones_ap = v_aug[b][:].rearrange("p (n d) -> p n d", d=DH1)[:, :, CH]
                nc.gpsimd.memset(ones_ap, 2.0)
            wv_cast_done = []

            def stage_v(b, kb):
                """v row-tile for keys [kb*128,(kb+1)*128) of batch b."""
                if "stagev" in ablate:
                    return
                cidx = (b * Q + kb * 128) // JC
                loc = (kb % KPC) * 128
                stage_xt_cast(cidx)
                if not wv_cast_done:
                    nc.vector.tensor_copy(wv_t[:], wv_st[:, 0 : CVB * HD])
                    wv_cast_done.append(True)
                xtb = xt_tiles[cidx]
                v_ps = ps512.tile([128, HD], F32, name="v_ps", tag="ps512")
                for cb in range(CVB):
                    nc.tensor.matmul(
                        v_ps[:],
                        xtb[:, cb * JC + loc : cb * JC + loc + 128],
                        wv_t[:, cb * HD : (cb + 1) * HD],
                        start=(cb == 0),
                        stop=(cb == CVB - 1),
                    )
                dst = v_aug[b][:].rearrange("p (kt h d) -> p kt h d", h=H, d=DH1)[
                    :, kb, :, 0:CH
                ]
                src = v_ps[:].rearrange("p (h d) -> p h d", d=CH)
                if kb % 2 == 0:
                    nc.vector.tensor_copy(dst, src)
                else:
                    nc.scalar.copy(dst, src)

            o_ps_cur = [None]

            def emit_epi(b, h, oT):
                """og_h = oT * (1+tanh)*recip (bf16), recip = 0.5/rowsum;
                then stream this head's two projection matmuls into o_ps."""
                if "epi" in ablate:
                    return
                rs = ep.tile([1, QL], F32, name="rs", tag="rs")
                nc.scalar.copy(rs[:], oT[CH : CH + 1, 0:QL])
                rsB = ep.tile([CH, QL], F32, name="rsB", tag="rsB")
                nc.gpsimd.partition_broadcast(rsB[:], rs[0:1, :])
                rT = ep.tile([CH, QL], F32, name="rT", tag="rT")
                nc.vector.reciprocal_approx_fast(rT[:], rsB[:])
                t1 = ep.tile([CH, QL], F32, name="t1", tag="t1")
                nc.vector.scalar_tensor_tensor(
                    t1[:], gt_t[:, h * QL : (h + 1) * QL], 1.0, rT[:],
                    ALU.add, ALU.mult,
                )
                nc.vector.tensor_mul(
                    og_t[b][:, h * QL : (h + 1) * QL], t1[:], oT[0:CH, 0:QL]
                )
                if b != B - 1:
                    return
                # streamed projection (last batch only: cuts the kernel tail)
                if h == 0:
                    o_ps_cur[0] = [
                        psO.tile([128, CV], F32, name=f"o_ps{qt}", tag=f"o{qt}")
                        for qt in range(QL // 128)
                    ]
                for qt in range(QL // 128):
                    nc.tensor.matmul(
                        o_ps_cur[0][qt][:],
                        og_t[b][:, h * QL + qt * 128 : h * QL + qt * 128 + 128],
                        wo_t[:, h * CV : (h + 1) * CV],
                        start=(h == 0),
                        stop=(h == H - 1),
                    )
                if h == H - 1:
                    for qt in range(QL // 128):
                        o_sb = op.tile([128, CV], F32, name="o_sb", tag="o_sb")
                        nc.vector.tensor_copy(o_sb[:], o_ps_cur[0][qt][:])
                        nc.scalar.dma_start(
                            out_d[b, qt * 128 : (qt + 1) * 128, :], o_sb[:]
                        )

            def emit_gates(b):
                """gates for all heads of b (transposed): tanh(0.5*Wg_h^T xT)"""
                if "gate" in ablate:
                    return
                for h in range(H):
                    gt_ps = psG.tile([CH, QL], F32, name="gt_ps", tag="gt_ps")
                    for cb in range(CVB):
                        nc.tensor.matmul(
                            gt_ps[:],
                            wg_t[:, cb * HD + h * CH : cb * HD + (h + 1) * CH],
                            xlt[:, cb * B * QL + b * QL : cb * B * QL + (b + 1) * QL],
                            start=(cb == 0),
                            stop=(cb == CVB - 1),
                        )
                    nc.scalar.activation(
                        gt_t[:, h * QL : (h + 1) * QL], gt_ps[:], AF.Tanh,
                        scale=0.5,
                    )

            def produce(b, h, halves=1):
                """b1 DMA + z-add + exp for head (b,h); returns sT."""
                b1t = b1p.tile([128, KQ], F32, name="b1t", tag="b1t")
                sT = sp.tile([128, KQ], BF16, name="sT", tag="sT")
                K2 = KQ // 2
                for chk in range(2):
                    if "b1dma" not in ablate:
                        # first head: chunk 1 rides the scalar queue so both
                        # halves land in parallel
                        eng = nc.scalar if (halves == 2 and chk == 1) else nc.sync
                        eng.dma_start(
                            b1t[:, chk * K2 : (chk + 1) * K2],
                            b1_d[b, h, :, chk * K2 : (chk + 1) * K2],
                        )
                    if halves == 2:
                        if "zadd" not in ablate:
                            nc.vector.tensor_add(
                                b1t[:, chk * K2 : (chk + 1) * K2],
                                b1t[:, chk * K2 : (chk + 1) * K2],
                                b2_t[:, b * KQ + chk * K2 : b * KQ + (chk + 1) * K2],
                            )
                        if "exp" not in ablate:
                            nc.scalar.activation(
                                sT[:, chk * K2 : (chk + 1) * K2],
                                b1t[:, chk * K2 : (chk + 1) * K2],
                                AF.Exp,
                            )
                if halves == 1:
                    if "zadd" not in ablate:
                        nc.vector.tensor_add(
                            b1t[:], b1t[:], b2_t[:, b * KQ : (b + 1) * KQ]
                        )
                    if "exp" not in ablate:
                        nc.scalar.activation(sT[:], b1t[:], AF.Exp)
                return sT

            # ---- main per-batch flow ----
            prestaged = [0]
            for b in range(B):
                pend = []
                sT_next = produce(b, 0, halves=2 if b == 0 else 1)
                for h in range(H):
                    # deferred epilogue first: its ACT/gpsimd/DVE ops must
                    # precede the next exp in each engine's in-order stream
                    if len(pend) >= 2:
                        ph, poT = pend.pop(0)
                        emit_epi(b, ph, poT)
                    if h == 1:
                        emit_gates(b)
                    if b == 0 and h == 1:
                        for ci in range(B * Q // JC // 2, B * Q // JC):
                            stage_xt_dma(ci)
                    if b == 0 and h == 2:
                        nc.gpsimd.dma_start(
                            wo_t[:].rearrange("p (hh c) -> p hh c", hh=H),
                            wo_d[:, :, :].rearrange("h p c -> p h c"),
                        )
                    if b == 0 and h == 4:
                        for ci in range(B * Q // JC // 2, 3 * B * Q // JC // 4):
                            stage_xt_cast(ci)
                    if b == 0 and h == 5:
                        for ci in range(3 * B * Q // JC // 4, B * Q // JC):
                            stage_xt_cast(ci)
                    if b == 0 and h == 5:
                        nc.sync.dma_start(b2_t[:, KQ : 2 * KQ], b2_d[1])

                    sT = sT_next
                    if h + 1 < H:
                        sT_next = produce(b, h + 1)

                    # attention: oT[d,q] (+2*rowsum via ones col) on TensorE
                    oT = psA.tile([DH1, 2 * QL], F32, name="oT", tag="oT")
                    if "attn" not in ablate:
                        for kb in range(KB):
                            if h == 0 and not (b == 1 and kb < prestaged[0]):
                                stage_v(b, kb)
                            # prestage next batch's v during b0's tail heads
                            if b == 0 and h >= 5 and kb % 3 == 0 and prestaged[0] < KB:
                                stage_v(1, prestaged[0])
                                prestaged[0] += 1
                            base = (kb * H + h) * DH1
                            nc.tensor.matmul(
                                oT[:, 0:QL],
                                v_aug[b][:, base : base + DH1],
                                sT[:, kb * QL : (kb + 1) * QL],
                                start=(kb == 0),
                                stop=(kb == KB - 1),
                            )

                    pend.append((h, oT))
                for ph, poT in pend:
                    emit_epi(b, ph, poT)
                if b != B - 1 and "epi" not in ablate:
                    # batch-end projection for non-final batches
                    for qt in range(QL // 128):
                        o_ps = ps512.tile([128, CV], F32, name="o_ps", tag="ps512")
                        for hh in range(H):
                            nc.tensor.matmul(
                                o_ps[:],
                                og_t[b][
                                    :, hh * QL + qt * 128 : hh * QL + qt * 128 + 128
                                ],
                                wo_t[:, hh * CV : (hh + 1) * CV],
                                start=(hh == 0),
                                stop=(hh == H - 1),
                            )
                        o_sb = op.tile([128, CV], F32, name="o_sb", tag="o_sb")
                        nc.vector.tensor_copy(o_sb[:], o_ps[:])
                        nc.scalar.dma_start(
                            out_d[b, qt * 128 : (qt + 1) * 128, :], o_sb[:]
                        )

    nc.compile()
    return nc


def make_in_maps(inputs, cfg=None):
    c = dict(CFG if cfg is None else cfg)
    B, Q, CV, NCORES, H, CH = c["B"], c["Q"], c["CV"], c["NCORES"], c["H"], c["CH"]
    HD = H * CH
    QL = Q // NCORES
    KB = Q // 128
    CVB = CV // 128
    x = np.ascontiguousarray(np.asarray(inputs["x"], dtype=np.float32))
    b1 = np.asarray(inputs["bias1"], dtype=np.float32)
    b2 = np.asarray(inputs["bias2"], dtype=np.float32)
    wv = np.ascontiguousarray(np.asarray(inputs["W_v"], dtype=np.float32)).reshape(
        CVB, 128, HD
    )
    wg = np.ascontiguousarray(np.asarray(inputs["W_g"], dtype=np.float32)).reshape(
        CVB, 128, HD
    )
    wo = np.ascontiguousarray(np.asarray(inputs["W_o"], dtype=np.float32)).reshape(
        H, CH, CV
    )
    # x^T: [CVB, 128, B*Q]
    xth = np.ascontiguousarray(x.reshape(B * Q, CV).T.reshape(CVB, 128, B * Q))
    # bias1 per-core: [B,H,128,KB*QL] with b1h[b,h,p,kb*QL+q]=b1[b,h,q0+q,kb*128+p]
    # one big transpose then per-core slices: [NC][B,H,128,KB,QL]
    b1g = np.ascontiguousarray(
        b1.reshape(B, H, NCORES, QL, KB, 128).transpose(2, 0, 1, 5, 4, 3)
    )
    b2g = np.ascontiguousarray(
        b2.reshape(B, NCORES, QL, KB, 128).transpose(1, 0, 4, 3, 2)
    )
    in_maps = []
    for cid in range(NCORES):
        sl = slice(cid * QL, (cid + 1) * QL)
        xlth = np.ascontiguousarray(
            x[:, sl, :].reshape(B * QL, CV).T.reshape(CVB, 128, B * QL)
        )
        in_maps.append(
            {
                "b1h": b1g[cid].reshape(B, H, 128, KB * QL),
                "b2h": b2g[cid].reshape(B, 128, KB * QL),
                "xth": xth,
                "xlth": xlth,
                "wvh": wv,
                "wgh": wg,
                "woh": wo,
            }
        )
    return in_maps


_NC_CACHE = {}


def kernel(**inputs) -> np.ndarray:
    key = "main"
    if key not in _NC_CACHE:
        _NC_CACHE[key] = build()
    nc = _NC_CACHE[key]
    in_maps = make_in_maps(inputs)
    res = bass_utils.run_bass_kernel_spmd(nc, in_maps, list(range(CFG["NCORES"])))
    outs = [res.results[cid]["out"] for cid in range(CFG["NCORES"])]
    return np.concatenate(outs, axis=1).astype(np.float32)


# revision 27
# speedup vs baseline: 1.0544x; 1.0544x over previous
"""BiasAttention Trainium2 Bass kernel (v3).

Computes, for x:[B,Q,CV], bias1:[B,H,Q,Q], bias2:[B,1,Q,Q], W_v/W_g:[CV,H*CH],
W_o:[H*CH,CV]:

    v = (x @ W_v) viewed [B,Q,H,CH]
    a = softmax(bias1 + bias2, axis=-1)
    o = einsum('bhqk,bhkd->bhqd', a, v) * sigmoid(x @ W_g)
    return o @ W_o

Sharding: data-parallel over the query dim Q across 8 NeuronCores (each core
computes QL=256 query rows end-to-end; no collectives).

v3 design notes (per core):
  - All host inputs are pre-arranged so every DMA lands 128 partitions with
    >=4KB contiguous per-partition runs (line-rate HBM DMA):
      b1h[b,h,p,kb*QL+q] = bias1[b,h,q0+q,kb*128+p]   (16KB/partition rows)
      b2h[b,p,kb*QL+q]   = bias2[b,0,q0+q,kb*128+p]
      xth[cb,p,j]        = x[j,cb*128+p]  (x transposed on host)
  - Startup-critical tensors (x^T, W_v, b2, b1) load f32 over the two HWDGE
    queues (sync + scalar); the v matmul runs f32r x f32r via bitcast (full
    rate at 512 free dim).  Only the small gate-path weights (wg, xl^T, wo)
    cast to bf16 through gpsimd SWDGE.
  - s = exp(b1+b2) in bf16 (attention matmul bf16 x bf16).
  - ScalarE runs ONLY Exp / Tanh / Copy (one ACT table set, no reloads).
    sigmoid(y) = 0.5*(1+tanh(y/2)); the 0.5 folds into the row-sum trick by
    setting the appended ones-column of v_aug to 2.0 (rowsum_psum = 2*sum(s),
    recip = 0.5/sum), and (1+tanh)*recip is one DVE scalar_tensor_tensor.
  - Row-sum reciprocal avoids single-partition DVE ops (1.75us each): ACT
    copies the PSUM row to SBUF (0.5us), gpsimd partition-broadcasts it to 64
    partitions, DVE takes the reciprocal wide.
  - Work is spread: z-adds mostly DVE with 1-in-4 on gpsimd; v_aug casts
    alternate DVE/ACT; per-head epilogues are emitted one head late so the
    in-order DVE stream never blocks the next head's add.
"""

import contextlib

import numpy as np


def _ensure_concourse():
    try:
        import concourse  # noqa: F401
    except ImportError:
        import sys

        for p in ("/root/.axon_site/_ro/trn_rl_repo", "/opt/trn_rl_repo"):
            if p not in sys.path:
                sys.path.insert(0, p)


_ensure_concourse()

import concourse.bacc as bacc  # noqa: E402
import concourse.mybir as mybir  # noqa: E402
import concourse.tile as tile  # noqa: E402
from concourse import bass_utils  # noqa: E402

F32 = mybir.dt.float32
F32R = mybir.dt.float32r
BF16 = mybir.dt.bfloat16
AF = mybir.ActivationFunctionType
ALU = mybir.AluOpType

# Problem dims (nn_BiasAttention): hardcoded per the harness contract.
CFG = dict(B=2, Q=2048, CV=512, H=8, CH=64, NCORES=8)


def build(cfg=None, repeat=1, ablate=()):
    c = dict(CFG if cfg is None else cfg)
    B, Q, CV, H, CH, NCORES = c["B"], c["Q"], c["CV"], c["H"], c["CH"], c["NCORES"]
    HD = H * CH
    QL = Q // NCORES  # query rows per core
    KB = Q // 128  # key blocks
    KQ = KB * QL  # free size of one (b,h) bias tile
    CVB = CV // 128
    DH1 = CH + 1  # head dim + ones column (row-sum trick)
    JC = 512  # x^T columns per staged chunk
    KPC = JC // 128  # key blocks covered per x^T chunk
    assert QL == 256 and CH == 64 and KQ == 4096

    nc = bacc.Bacc("TRN2", target_bir_lowering=False, debug=False, num_devices=NCORES)

    b1_d = nc.dram_tensor("b1h", [B, H, 128, KQ], F32, kind="ExternalInput")
    b2_d = nc.dram_tensor("b2h", [B, 128, KQ], F32, kind="ExternalInput")
    xt_d = nc.dram_tensor("xth", [CVB, 128, B * Q], F32, kind="ExternalInput")
    xlt_d = nc.dram_tensor("xlth", [CVB, 128, B * QL], F32, kind="ExternalInput")
    wv_d = nc.dram_tensor("wvh", [CVB, 128, HD], F32, kind="ExternalInput")
    wg_d = nc.dram_tensor("wgh", [CVB, 128, HD], F32, kind="ExternalInput")
    wo_d = nc.dram_tensor("woh", [H, CH, CV], F32, kind="ExternalInput")
    out_d = nc.dram_tensor("out", [B, QL, CV], F32, kind="ExternalOutput")

    with tile.TileContext(nc) as tc:
        loop = tc.For_i(0, repeat, 1) if repeat > 1 else contextlib.nullcontext()
        with loop, contextlib.ExitStack() as ctx:
            persist = ctx.enter_context(tc.tile_pool(name="persist", bufs=1))
            b1p = ctx.enter_context(tc.tile_pool(name="b1p", bufs=2))
            sp = ctx.enter_context(tc.tile_pool(name="sp", bufs=3))
            xp = ctx.enter_context(tc.tile_pool(name="xp", bufs=3))
            xbp = ctx.enter_context(tc.tile_pool(name="xbp", bufs=3))
            ep = ctx.enter_context(tc.tile_pool(name="ep", bufs=2))
            op = ctx.enter_context(tc.tile_pool(name="op", bufs=2))
            psA = ctx.enter_context(tc.tile_pool(name="psA", bufs=4, space="PSUM"))
            ps512 = ctx.enter_context(tc.tile_pool(name="ps512", bufs=2, space="PSUM"))
            psG = ctx.enter_context(tc.tile_pool(name="psG", bufs=2, space="PSUM"))

            # ---- persistent tiles ----
            b2_t = persist.tile([128, B * KQ], F32, name="b2_t", tag="b2_t")
            v_aug = [
                persist.tile([128, KB * H * DH1], BF16, name=f"vaug{b}", tag=f"vaug{b}")
                for b in range(B)
            ]
            xlt = persist.tile([128, CVB * B * QL], BF16, name="xlt", tag="xlt")
            wv_t = persist.tile([128, CVB * HD], BF16, name="wv_t", tag="wv_t")
            wg_t = persist.tile([128, CVB * HD], BF16, name="wg_t", tag="wg_t")
            wo_t = persist.tile([CH, H * CV], BF16, name="wo_t", tag="wo_t")
            gt_t = persist.tile([CH, H * QL], F32, name="gt_t", tag="gt_t")
            og_shared = persist.tile([CH, H * QL], BF16, name="og", tag="og")
            og_t = [og_shared for _ in range(B)]

            # ---- setup DMAs ----
            # PE warm-up source: memset a scratch strip of v_aug[0]; ~36
            # accumulating matmuls on it keep the HAM activity window busy so
            # the PE clock is at 2.4GHz when real v-staging matmuls arrive.
            nc.gpsimd.memset(v_aug[0][:, 0 : 2 * QL + DH1], 1.0)
            # b2[0] split across both HWDGE queues for startup latency
            nc.sync.dma_start(b2_t[:, 0 : KQ // 2], b2_d[0][:, 0 : KQ // 2])
            xt_tiles = {}

            xt_f32 = {}

            def stage_xt_dma(cidx, eng=None):
                """x^T chunk: f32 load (scalar HWDGE or gpsimd SWDGE)."""
                if cidx in xt_f32:
                    return
                eng = eng or nc.scalar
                xt = xp.tile([128, CVB * JC], F32, name=f"xt{cidx}", tag="xt")
                eng.dma_start(
                    xt[:].rearrange("p (cb j) -> p cb j", cb=CVB),
                    xt_d[:, :, cidx * JC : (cidx + 1) * JC].rearrange(
                        "cb p j -> p cb j"
                    ),
                )
                xt_f32[cidx] = xt

            def stage_xt_cast(cidx):
                """bf16 cast on DVE (even chunks) / ACT (odd chunks)."""
                if cidx in xt_tiles:
                    return
                stage_xt_dma(cidx)
                xtb = xbp.tile([128, CVB * JC], BF16, name=f"xtb{cidx}", tag="xtb")
                if cidx % 2 == 0:
                    nc.vector.tensor_copy(xtb[:], xt_f32[cidx][:])
                else:
                    nc.scalar.copy(xtb[:], xt_f32[cidx][:])
                xt_tiles[cidx] = xtb

            # wv staged f32 through an xt-pool slot (reused by later chunks);
            # it leads the scalar queue so it lands early
            wv_st = xp.tile([128, CVB * JC], F32, name="wv_st", tag="xt")
            nc.scalar.dma_start(
                wv_st[:, 0 : CVB * HD].rearrange("p (cb d) -> p cb d", cb=CVB),
                wv_d[:, :, :].rearrange("cb p d -> p cb d"),
            )
            nc.scalar.dma_start(
                b2_t[:, KQ // 2 : KQ], b2_d[0][:, KQ // 2 : KQ]
            )
            # x^T chunk 0 leads the gpsimd queue; gate weights interleave
            stage_xt_dma(0, nc.gpsimd)
            nc.gpsimd.dma_start(
                wg_t[:].rearrange("p (cb d) -> p cb d", cb=CVB),
                wg_d[:, :, :].rearrange("cb p d -> p cb d"),
            )
            stage_xt_dma(1, nc.gpsimd)
            nc.gpsimd.dma_start(
                xlt[:].rearrange("p (cb j) -> p cb j", cb=CVB),
                xlt_d[:, :, :].rearrange("cb p j -> p cb j"),
            )
            # warm-up matmuls (results discarded; psA slot recycled later)
            warm_ps = psA.tile([DH1, 2 * QL], F32, name="warm", tag="oT")
            for i in range(36):
                nc.tensor.matmul(
                    warm_ps[:, 0:QL],
                    v_aug[0][:, 0:DH1],
                    v_aug[0][:, DH1 : DH1 + QL],
                    start=(i == 0),
                    stop=(i == 35),
                )
            # ones columns (value 2.0: folds the 0.5 of the tanh-sigmoid)
            for b in range(B):
                ones_ap = v_aug[b][:].rearrange("p (n d) -> p n d", d=DH1)[:, :, CH]
                nc.gpsimd.memset(ones_ap, 2.0)
            wv_cast_done = []

            def stage_v(b, kb):
                """v row-tile for keys [kb*128,(kb+1)*128) of batch b."""
                if "stagev" in ablate:
                    return
                cidx = (b * Q + kb * 128) // JC
                loc = (kb % KPC) * 128
                stage_xt_cast(cidx)
                if not wv_cast_done:
                    nc.vector.tensor_copy(wv_t[:], wv_st[:, 0 : CVB * HD])
                    wv_cast_done.append(True)
                xtb = xt_tiles[cidx]
                v_ps = ps512.tile([128, HD], F32, name="v_ps", tag="ps512")
                for cb in range(CVB):
                    nc.tensor.matmul(
                        v_ps[:],
                        xtb[:, cb * JC + loc : cb * JC + loc + 128],
                        wv_t[:, cb * HD : (cb + 1) * HD],
                        start=(cb == 0),
                        stop=(cb == CVB - 1),
                    )
                dst = v_aug[b][:].rearrange("p (kt h d) -> p kt h d", h=H, d=DH1)[
                    :, kb, :, 0:CH
                ]
                src = v_ps[:].rearrange("p (h d) -> p h d", d=CH)
                if kb % 2 == 0:
                    nc.vector.tensor_copy(dst, src)
                else:
                    nc.scalar.copy(dst, src)

            o_ps_cur = [None]

            def emit_epi(b, h, oT):
                """og_h = oT * (1+tanh)*recip (bf16), recip = 0.5/rowsum;
                then stream this head's two projection matmuls into o_ps."""
                if "epi" in ablate:
                    return
                rs = ep.tile([1, QL], F32, name="rs", tag="rs")
                nc.scalar.copy(rs[:], oT[CH : CH + 1, 0:QL])
                rsB = ep.tile([CH, QL], F32, name="rsB", tag="rsB")
                nc.gpsimd.partition_broadcast(rsB[:], rs[0:1, :])
                rT = ep.tile([CH, QL], F32, name="rT", tag="rT")
                nc.vector.reciprocal_approx_fast(rT[:], rsB[:])
                t1 = ep.tile([CH, QL], F32, name="t1", tag="t1")
                nc.vector.scalar_tensor_tensor(
                    t1[:], gt_t[:, h * QL : (h + 1) * QL], 1.0, rT[:],
                    ALU.add, ALU.mult,
                )
                nc.vector.tensor_mul(
                    og_t[b][:, h * QL : (h + 1) * QL], t1[:], oT[0:CH, 0:QL]
                )

            def emit_gates(b):
                """gates for all heads of b (transposed): tanh(0.5*Wg_h^T xT)"""
                if "gate" in ablate:
                    return
                for h in range(H):
                    gt_ps = psG.tile([CH, QL], F32, name="gt_ps", tag="gt_ps")
                    for cb in range(CVB):
                        nc.tensor.matmul(
                            gt_ps[:],
                            wg_t[:, cb * HD + h * CH : cb * HD + (h + 1) * CH],
                            xlt[:, cb * B * QL + b * QL : cb * B * QL + (b + 1) * QL],
                            start=(cb == 0),
                            stop=(cb == CVB - 1),
                        )
                    nc.scalar.activation(
                        gt_t[:, h * QL : (h + 1) * QL], gt_ps[:], AF.Tanh,
                        scale=0.5,
                    )

            def produce(b, h, halves=1):
                """b1 DMA + z-add + exp for head (b,h); returns sT."""
                b1t = b1p.tile([128, KQ], F32, name="b1t", tag="b1t")
                sT = sp.tile([128, KQ], BF16, name="sT", tag="sT")
                K2 = KQ // 2
                for chk in range(2):
                    if "b1dma" not in ablate:
                        # first head: chunk 1 rides the scalar queue so both
                        # halves land in parallel
                        eng = nc.scalar if (halves == 2 and chk == 1) else nc.sync
                        eng.dma_start(
                            b1t[:, chk * K2 : (chk + 1) * K2],
                            b1_d[b, h, :, chk * K2 : (chk + 1) * K2],
                        )
                    if halves == 2:
                        if "zadd" not in ablate:
                            nc.vector.tensor_add(
                                b1t[:, chk * K2 : (chk + 1) * K2],
                                b1t[:, chk * K2 : (chk + 1) * K2],
                                b2_t[:, b * KQ + chk * K2 : b * KQ + (chk + 1) * K2],
                            )
                        if "exp" not in ablate:
                            nc.scalar.activation(
                                sT[:, chk * K2 : (chk + 1) * K2],
                                b1t[:, chk * K2 : (chk + 1) * K2],
                                AF.Exp,
                            )
                if halves == 1:
                    if "zadd" not in ablate:
                        nc.vector.tensor_add(
                            b1t[:], b1t[:], b2_t[:, b * KQ : (b + 1) * KQ]
                        )
                    if "exp" not in ablate:
                        nc.scalar.activation(sT[:], b1t[:], AF.Exp)
                return sT

            # ---- main per-batch flow ----
            prestaged = [0]
            for b in range(B):
                pend = []
                sT_next = produce(b, 0, halves=2 if b == 0 else 1)
                for h in range(H):
                    # deferred epilogue first: its ACT/gpsimd/DVE ops must
                    # precede the next exp in each engine's in-order stream
                    if len(pend) >= 2:
                        ph, poT = pend.pop(0)
                        emit_epi(b, ph, poT)
                    if h == 1:
                        emit_gates(b)
                    if b == 0 and h == 1:
                        for ci in range(B * Q // JC // 2, B * Q // JC):
                            stage_xt_dma(ci)
                    if b == 0 and h == 2:
                        nc.gpsimd.dma_start(
                            wo_t[:].rearrange("p (hh c) -> p hh c", hh=H),
                            wo_d[:, :, :].rearrange("h p c -> p h c"),
                        )
                    if b == 0 and h == 4:
                        for ci in range(B * Q // JC // 2, 3 * B * Q // JC // 4):
                            stage_xt_cast(ci)
                    if b == 0 and h == 5:
                        for ci in range(3 * B * Q // JC // 4, B * Q // JC):
                            stage_xt_cast(ci)
                    if b == 0 and h == 5:
                        nc.sync.dma_start(b2_t[:, KQ : 2 * KQ], b2_d[1])

                    sT = sT_next
                    if h + 1 < H:
                        sT_next = produce(b, h + 1)

                    # attention: oT[d,q] (+2*rowsum via ones col) on TensorE
                    oT = psA.tile([DH1, 2 * QL], F32, name="oT", tag="oT")
                    if "attn" not in ablate:
                        for kb in range(KB):
                            if h == 0 and not (b == 1 and kb < prestaged[0]):
                                stage_v(b, kb)
                            # prestage next batch's v during b0's tail heads
                            if b == 0 and h >= 5 and kb % 3 == 0 and prestaged[0] < KB:
                                stage_v(1, prestaged[0])
                                prestaged[0] += 1
                            base = (kb * H + h) * DH1
                            nc.tensor.matmul(
                                oT[:, 0:QL],
                                v_aug[b][:, base : base + DH1],
                                sT[:, kb * QL : (kb + 1) * QL],
                                start=(kb == 0),
                                stop=(kb == KB - 1),
                            )

                    pend.append((h, oT))
                if "epi" not in ablate:
                    o_pss = [
                        ps512.tile([128, CV], F32, name=f"o_ps{qt}", tag="ps512")
                        for qt in range(QL // 128)
                    ]

                    def proj(hh, qt):
                        nc.tensor.matmul(
                            o_pss[qt][:],
                            og_t[b][:, hh * QL + qt * 128 : hh * QL + qt * 128 + 128],
                            wo_t[:, hh * CV : (hh + 1) * CV],
                            start=(hh == 0),
                            stop=(hh == H - 1),
                        )

                    # epi_6, then heads 0..6 projected while epi_7 finishes
                    emit_epi(b, pend[0][0], pend[0][1])
                    for qt in range(QL // 128):
                        for hh in range(H - 1):
                            proj(hh, qt)
                    emit_epi(b, pend[1][0], pend[1][1])
                    for qt in range(QL // 128):
                        proj(H - 1, qt)
                        o_sb = op.tile([128, CV], F32, name="o_sb", tag="o_sb")
                        nc.vector.tensor_copy(o_sb[:], o_pss[qt][:])
                        nc.scalar.dma_start(
                            out_d[b, qt * 128 : (qt + 1) * 128, :], o_sb[:]
                        )
                else:
                    for ph, poT in pend:
                        emit_epi(b, ph, poT)

    nc.compile()
    return nc


def make_in_maps(inputs, cfg=None):
    c = dict(CFG if cfg is None else cfg)
    B, Q, CV, NCORES, H, CH = c["B"], c["Q"], c["CV"], c["NCORES"], c["H"], c["CH"]
    HD = H * CH
    QL = Q // NCORES
    KB = Q // 128
    CVB = CV // 128
    x = np.ascontiguousarray(np.asarray(inputs["x"], dtype=np.float32))
    b1 = np.asarray(inputs["bias1"], dtype=np.float32)
    b2 = np.asarray(inputs["bias2"], dtype=np.float32)
    wv = np.ascontiguousarray(np.asarray(inputs["W_v"], dtype=np.float32)).reshape(
        CVB, 128, HD
    )
    wg = np.ascontiguousarray(np.asarray(inputs["W_g"], dtype=np.float32)).reshape(
        CVB, 128, HD
    )
    wo = np.ascontiguousarray(np.asarray(inputs["W_o"], dtype=np.float32)).reshape(
        H, CH, CV
    )
    # x^T: [CVB, 128, B*Q]
    xth = np.ascontiguousarray(x.reshape(B * Q, CV).T.reshape(CVB, 128, B * Q))
    # bias1 per-core: [B,H,128,KB*QL] with b1h[b,h,p,kb*QL+q]=b1[b,h,q0+q,kb*128+p]
    # one big transpose then per-core slices: [NC][B,H,128,KB,QL]
    b1g = np.ascontiguousarray(
        b1.reshape(B, H, NCORES, QL, KB, 128).transpose(2, 0, 1, 5, 4, 3)
    )
    b2g = np.ascontiguousarray(
        b2.reshape(B, NCORES, QL, KB, 128).transpose(1, 0, 4, 3, 2)
    )
    in_maps = []
    for cid in range(NCORES):
        sl = slice(cid * QL, (cid + 1) * QL)
        xlth = np.ascontiguousarray(
            x[:, sl, :].reshape(B * QL, CV).T.reshape(CVB, 128, B * QL)
        )
        in_maps.append(
            {
                "b1h": b1g[cid].reshape(B, H, 128, KB * QL),
                "b2h": b2g[cid].reshape(B, 128, KB * QL),
                "xth": xth,
                "xlth": xlth,
                "wvh": wv,
                "wgh": wg,
                "woh": wo,
            }
        )
    return in_maps


_NC_CACHE = {}


def kernel(**inputs) -> np.ndarray:
    key = "main"
    if key not in _NC_CACHE:
        _NC_CACHE[key] = build()
    nc = _NC_CACHE[key]
    in_maps = make_in_maps(inputs)
    res = bass_utils.run_bass_kernel_spmd(nc, in_maps, list(range(CFG["NCORES"])))
    outs = [res.results[cid]["out"] for cid in range(CFG["NCORES"])]
    return np.concatenate(outs, axis=1).astype(np.float32)


# revision 29
# speedup vs baseline: 1.5273x; 1.4485x over previous
"""BiasAttention Trainium2 Bass kernel (v7, bf16 inputs).

Computes, for x:[B,Q,CV], bias1:[B,H,Q,Q], bias2:[B,1,Q,Q], W_v/W_g:[CV,H*CH],
W_o:[H*CH,CV]:

    v = (x @ W_v) viewed [B,Q,H,CH]
    a = softmax(bias1 + bias2, axis=-1)
    o = einsum('bhqk,bhkd->bhqd', a, v) * sigmoid(x @ W_g)
    return o @ W_o

Sharding: data-parallel over the query dim Q across 8 NeuronCores (each core
computes QL=256 query rows end-to-end; no collectives).

Design notes (per core):
  - Inputs are marshalled on the host: per-core slices, transposed so every
    DMA lands 128 partitions with large contiguous per-partition runs, and
    cast to bf16 (the kernel's compute precision; tolerance is 2e-2):
      b1h[b,h,p,kb*QL+q] = bias1[b,h,q0+q,kb*128+p]
      b2h[b,p,kb*QL+q]   = bias2[b,0,q0+q,kb*128+p]
      xth[cb,p,j]        = x[j,cb*128+p]
    This halves HBM traffic (~26MB/core) vs f32 and enables 2x-rate DVE adds.
  - z = b1+b2 in-place on DVE (all-bf16), s = exp(z) on ScalarE (bf16 out),
    attention oT[d,q] accumulates on TensorE per head with a ones-column
    (value 2.0) appended to v for row sums.
  - ScalarE runs ONLY Exp / Tanh / Copy (one ACT table set, no reloads).
    sigmoid(y) = 0.5*(1+tanh(y/2)); the 0.5 folds into the 2.0 ones-column
    (recip = 0.5/sum) and (1+tanh)*recip is one DVE scalar_tensor_tensor.
  - Row-sum reciprocal avoids slow single-partition DVE ops: ACT copies the
    PSUM row to SBUF, gpsimd partition-broadcasts to 64 partitions, DVE runs
    reciprocal_approx_fast wide.
  - Gate is computed transposed (gT = Wg_h^T @ xT) per batch at h==1; the
    per-head epilogue is emitted at the TOP of the iteration two heads later
    so the in-order engine streams never put attention-dependent epilogue
    work ahead of the next head's add/exp.
  - PE warm-up matmuls at t~8us hold the HAM activity window so v-staging
    runs at 2.4GHz; v for batch 1 is prestaged during batch 0's tail heads.
  - The final batch's projection is split: heads 0..6 matmul after epi_6 so
    the kernel tail is only the last head's chain.
"""

import contextlib

import numpy as np


def _ensure_concourse():
    try:
        import concourse  # noqa: F401
    except ImportError:
        import sys

        for p in ("/root/.axon_site/_ro/trn_rl_repo", "/opt/trn_rl_repo"):
            if p not in sys.path:
                sys.path.insert(0, p)


_ensure_concourse()

import ml_dtypes  # noqa: E402
import concourse.bacc as bacc  # noqa: E402
import concourse.mybir as mybir  # noqa: E402
import concourse.tile as tile  # noqa: E402
from concourse import bass_utils  # noqa: E402

F32 = mybir.dt.float32
BF16 = mybir.dt.bfloat16
NP_BF16 = ml_dtypes.bfloat16
AF = mybir.ActivationFunctionType
ALU = mybir.AluOpType

# Problem dims (nn_BiasAttention): hardcoded per the harness contract.
CFG = dict(B=2, Q=2048, CV=512, H=8, CH=64, NCORES=8)


def build(cfg=None, repeat=1, ablate=()):
    c = dict(CFG if cfg is None else cfg)
    B, Q, CV, H, CH, NCORES = c["B"], c["Q"], c["CV"], c["H"], c["CH"], c["NCORES"]
    HD = H * CH
    QL = Q // NCORES  # query rows per core
    KB = Q // 128  # key blocks
    KQ = KB * QL  # free size of one (b,h) bias tile
    CVB = CV // 128
    DH1 = CH + 1  # head dim + ones column (row-sum trick)
    JC = 512  # x^T columns per staged chunk
    KPC = JC // 128  # key blocks covered per x^T chunk
    NCH = B * Q // JC  # number of x^T chunks
    assert QL == 256 and CH == 64 and KQ == 4096

    nc = bacc.Bacc("TRN2", target_bir_lowering=False, debug=False, num_devices=NCORES)

    b1_d = nc.dram_tensor("b1h", [B, H, 128, KQ], BF16, kind="ExternalInput")
    b2_d = nc.dram_tensor("b2h", [B, 128, KQ], BF16, kind="ExternalInput")
    xt_d = nc.dram_tensor("xth", [CVB, 128, B * Q], BF16, kind="ExternalInput")
    xlt_d = nc.dram_tensor("xlth", [CVB, 128, B * QL], BF16, kind="ExternalInput")
    wv_d = nc.dram_tensor("wvh", [CVB, 128, HD], BF16, kind="ExternalInput")
    wg_d = nc.dram_tensor("wgh", [CVB, 128, HD], BF16, kind="ExternalInput")
    wo_d = nc.dram_tensor("woh", [H, CH, CV], BF16, kind="ExternalInput")
    out_d = nc.dram_tensor("out", [B, QL, CV], F32, kind="ExternalOutput")

    with tile.TileContext(nc) as tc:
        loop = tc.For_i(0, repeat, 1) if repeat > 1 else contextlib.nullcontext()
        with loop, contextlib.ExitStack() as ctx:
            persist = ctx.enter_context(tc.tile_pool(name="persist", bufs=1))
            b1p = ctx.enter_context(tc.tile_pool(name="b1p", bufs=4))
            sp = ctx.enter_context(tc.tile_pool(name="sp", bufs=4))
            xbp = ctx.enter_context(tc.tile_pool(name="xbp", bufs=8))
            ep = ctx.enter_context(tc.tile_pool(name="ep", bufs=3))
            op = ctx.enter_context(tc.tile_pool(name="op", bufs=2))
            psA = ctx.enter_context(tc.tile_pool(name="psA", bufs=4, space="PSUM"))
            ps512 = ctx.enter_context(tc.tile_pool(name="ps512", bufs=2, space="PSUM"))
            psG = ctx.enter_context(tc.tile_pool(name="psG", bufs=2, space="PSUM"))

            # ---- persistent tiles ----
            b2_t = persist.tile([128, B * KQ], BF16, name="b2_t", tag="b2_t")
            v_aug = [
                persist.tile([128, KB * H * DH1], BF16, name=f"vaug{b}", tag=f"vaug{b}")
                for b in range(B)
            ]
            xlt = persist.tile([128, CVB * B * QL], BF16, name="xlt", tag="xlt")
            wv_t = persist.tile([128, CVB * HD], BF16, name="wv_t", tag="wv_t")
            wg_t = persist.tile([128, CVB * HD], BF16, name="wg_t", tag="wg_t")
            wo_t = persist.tile([CH, H * CV], BF16, name="wo_t", tag="wo_t")
            gt_t = persist.tile([CH, H * QL], F32, name="gt_t", tag="gt_t")
            og_t = persist.tile([CH, H * QL], BF16, name="og", tag="og")

            # ---- setup ----
            # PE warm-up source: scratch strip of v_aug[0]
            nc.gpsimd.memset(v_aug[0][:, 0 : QL + DH1], 1.0)
            # b2[0] split across both HWDGE queues for startup latency
            nc.sync.dma_start(b2_t[:, 0 : KQ // 2], b2_d[0][:, 0 : KQ // 2])
            nc.scalar.dma_start(
                wv_t[:].rearrange("p (cb d) -> p cb d", cb=CVB),
                wv_d[:, :, :].rearrange("cb p d -> p cb d"),
            )
            nc.scalar.dma_start(
                b2_t[:, KQ // 2 : KQ], b2_d[0][:, KQ // 2 : KQ]
            )
            xt_tiles = {}

            def stage_xt(cidx, eng=None):
                if cidx in xt_tiles:
                    return
                eng = eng or nc.scalar
                xt = xbp.tile([128, CVB * JC], BF16, name=f"xt{cidx}", tag="xt")
                eng.dma_start(
                    xt[:].rearrange("p (cb j) -> p cb j", cb=CVB),
                    xt_d[:, :, cidx * JC : (cidx + 1) * JC].rearrange(
                        "cb p j -> p cb j"
                    ),
                )
                xt_tiles[cidx] = xt

            # b0's x^T chunks + gate weights interleave on the gpsimd queue
            stage_xt(0, nc.gpsimd)
            nc.gpsimd.dma_start(
                wg_t[:].rearrange("p (cb d) -> p cb d", cb=CVB),
                wg_d[:, :, :].rearrange("cb p d -> p cb d"),
            )
            stage_xt(1, nc.gpsimd)
            nc.gpsimd.dma_start(
                xlt[:].rearrange("p (cb j) -> p cb j", cb=CVB),
                xlt_d[:, :, :].rearrange("cb p j -> p cb j"),
            )
            stage_xt(2, nc.gpsimd)
            stage_xt(3, nc.gpsimd)
            # warm-up matmuls (results discarded; psA slot recycled later)
            warm_ps = psA.tile([DH1, 2 * QL], F32, name="warm", tag="oT")
            for i in range(36):
                nc.tensor.matmul(
                    warm_ps[:, 0:QL],
                    v_aug[0][:, 0:DH1],
                    v_aug[0][:, DH1 : DH1 + QL],
                    start=(i == 0),
                    stop=(i == 35),
                )
            # ones columns (value 2.0: folds the 0.5 of the tanh-sigmoid)
            for b in range(B):
                ones_ap = v_aug[b][:].rearrange("p (n d) -> p n d", d=DH1)[:, :, CH]
                nc.gpsimd.memset(ones_ap, 2.0)

            def stage_v(b, kb):
                """v row-tile for keys [kb*128,(kb+1)*128) of batch b."""
                if "stagev" in ablate:
                    return
                cidx = (b * Q + kb * 128) // JC
                loc = (kb % KPC) * 128
                stage_xt(cidx)
                xtb = xt_tiles[cidx]
                v_ps = ps512.tile([128, HD], F32, name="v_ps", tag="ps512")
                for cb in range(CVB):
                    nc.tensor.matmul(
                        v_ps[:],
                        xtb[:, cb * JC + loc : cb * JC + loc + 128],
                        wv_t[:, cb * HD : (cb + 1) * HD],
                        start=(cb == 0),
                        stop=(cb == CVB - 1),
                    )
                dst = v_aug[b][:].rearrange("p (kt h d) -> p kt h d", h=H, d=DH1)[
                    :, kb, :, 0:CH
                ]
                src = v_ps[:].rearrange("p (h d) -> p h d", d=CH)
                if kb % 2 == 0:
                    nc.vector.tensor_copy(dst, src)
                else:
                    nc.scalar.copy(dst, src)

            def emit_epi(b, h, oT):
                """og_h = oT * (1+tanh)*recip  (bf16), recip = 0.5/rowsum."""
                if "epi" in ablate:
                    return
                rs = ep.tile([1, QL], F32, name="rs", tag="rs")
                nc.scalar.copy(rs[:], oT[CH : CH + 1, 0:QL])
                rsB = ep.tile([CH, QL], F32, name="rsB", tag="rsB")
                nc.gpsimd.partition_broadcast(rsB[:], rs[0:1, :])
                rT = ep.tile([CH, QL], F32, name="rT", tag="rT")
                nc.vector.reciprocal_approx_fast(rT[:], rsB[:])
                t1 = ep.tile([CH, QL], F32, name="t1", tag="t1")
                nc.vector.scalar_tensor_tensor(
                    t1[:], gt_t[:, h * QL : (h + 1) * QL], 1.0, rT[:],
                    ALU.add, ALU.mult,
                )
                nc.vector.tensor_mul(
                    og_t[:, h * QL : (h + 1) * QL], t1[:], oT[0:CH, 0:QL]
                )

            def emit_gates(b):
                """gates for all heads of b (transposed): tanh(0.5*Wg_h^T xT)"""
                if "gate" in ablate:
                    return
                for h in range(H):
                    gt_ps = psG.tile([CH, QL], F32, name="gt_ps", tag="gt_ps")
                    for cb in range(CVB):
                        nc.tensor.matmul(
                            gt_ps[:],
                            wg_t[:, cb * HD + h * CH : cb * HD + (h + 1) * CH],
                            xlt[:, cb * B * QL + b * QL : cb * B * QL + (b + 1) * QL],
                            start=(cb == 0),
                            stop=(cb == CVB - 1),
                        )
                    nc.scalar.activation(
                        gt_t[:, h * QL : (h + 1) * QL], gt_ps[:], AF.Tanh,
                        scale=0.5,
                    )

            def produce(b, h, halves=1):
                """b1 DMA + z-add + exp for head (b,h); returns sT."""
                b1t = b1p.tile([128, KQ], BF16, name="b1t", tag="b1t")
                sT = sp.tile([128, KQ], BF16, name="sT", tag="sT")
                K2 = KQ // 2
                for chk in range(2):
                    if "b1dma" not in ablate:
                        eng = nc.scalar if (halves == 2 and chk == 1) else nc.sync
                        eng.dma_start(
                            b1t[:, chk * K2 : (chk + 1) * K2],
                            b1_d[b, h, :, chk * K2 : (chk + 1) * K2],
                        )
                    if halves == 2:
                        if "zadd" not in ablate:
                            nc.vector.tensor_add(
                                b1t[:, chk * K2 : (chk + 1) * K2],
                                b1t[:, chk * K2 : (chk + 1) * K2],
                                b2_t[:, b * KQ + chk * K2 : b * KQ + (chk + 1) * K2],
                            )
                        if "exp" not in ablate:
                            nc.scalar.activation(
                                sT[:, chk * K2 : (chk + 1) * K2],
                                b1t[:, chk * K2 : (chk + 1) * K2],
                                AF.Exp,
                            )
                if halves == 1:
                    if "zadd" not in ablate:
                        nc.vector.tensor_add(
                            b1t[:], b1t[:], b2_t[:, b * KQ : (b + 1) * KQ]
                        )
                    if "exp" not in ablate:
                        nc.scalar.activation(sT[:], b1t[:], AF.Exp)
                return sT

            # ---- main per-batch flow ----
            prestaged = [0]
            for b in range(B):
                pend = []
                sT_next = produce(b, 0, halves=2 if b == 0 else 1)
                for h in range(H):
                    # deferred epilogue first: its ACT/gpsimd/DVE ops must
                    # precede the next exp in each engine's in-order stream
                    if len(pend) >= 2:
                        ph, poT = pend.pop(0)
                        emit_epi(b, ph, poT)
                    if h == 1:
                        emit_gates(b)
                    if b == 0 and h == 1:
                        for ci in range(NCH // 2, NCH):
                            stage_xt(ci)
                    if b == 0 and h == 2:
                        nc.gpsimd.dma_start(
                            wo_t[:].rearrange("p (hh cc) -> p hh cc", hh=H),
                            wo_d[:, :, :].rearrange("h p c -> p h c"),
                        )
                    if b == 0 and h == 5:
                        nc.scalar.dma_start(b2_t[:, KQ : 2 * KQ], b2_d[1])

                    sT = sT_next
                    if h + 1 < H:
                        sT_next = produce(b, h + 1)

                    # attention: oT[d,q] (+2*rowsum via ones col) on TensorE
                    oT = psA.tile([DH1, 2 * QL], F32, name="oT", tag="oT")
                    if "attn" not in ablate:
                        for kb in range(KB):
                            if h == 0 and not (b == 1 and kb < prestaged[0]):
                                stage_v(b, kb)
                            # prestage next batch's v during b0's tail heads
                            if b == 0 and h >= 5 and kb % 3 == 0 and prestaged[0] < KB:
                                stage_v(1, prestaged[0])
                                prestaged[0] += 1
                            base = (kb * H + h) * DH1
                            nc.tensor.matmul(
                                oT[:, 0:QL],
                                v_aug[b][:, base : base + DH1],
                                sT[:, kb * QL : (kb + 1) * QL],
                                start=(kb == 0),
                                stop=(kb == KB - 1),
                            )

                    pend.append((h, oT))

                if "epi" in ablate:
                    continue
                if b != B - 1:
                    for ph, poT in pend:
                        emit_epi(b, ph, poT)
                    for qt in range(QL // 128):
                        o_ps = ps512.tile([128, CV], F32, name="o_ps", tag="ps512")
                        for hh in range(H):
                            nc.tensor.matmul(
                                o_ps[:],
                                og_t[:, hh * QL + qt * 128 : hh * QL + qt * 128 + 128],
                                wo_t[:, hh * CV : (hh + 1) * CV],
                                start=(hh == 0),
                                stop=(hh == H - 1),
                            )
                        o_sb = op.tile([128, CV], F32, name="o_sb", tag="o_sb")
                        nc.vector.tensor_copy(o_sb[:], o_ps[:])
                        nc.scalar.dma_start(
                            out_d[b, qt * 128 : (qt + 1) * 128, :], o_sb[:]
                        )
                else:
                    # split tail: epi_6, project heads 0..6, epi_7, finish
                    o_pss = [
                        ps512.tile([128, CV], F32, name=f"o_ps{qt}", tag="ps512")
                        for qt in range(QL // 128)
                    ]

                    def proj(hh, qt):
                        nc.tensor.matmul(
                            o_pss[qt][:],
                            og_t[:, hh * QL + qt * 128 : hh * QL + qt * 128 + 128],
                            wo_t[:, hh * CV : (hh + 1) * CV],
                            start=(hh == 0),
                            stop=(hh == H - 1),
                        )

                    emit_epi(b, pend[0][0], pend[0][1])
                    for qt in range(QL // 128):
                        for hh in range(H - 1):
                            proj(hh, qt)
                    emit_epi(b, pend[1][0], pend[1][1])
                    for qt in range(QL // 128):
                        proj(H - 1, qt)
                        o_sb = op.tile([128, CV], F32, name="o_sb", tag="o_sb")
                        nc.vector.tensor_copy(o_sb[:], o_pss[qt][:])
                        nc.scalar.dma_start(
                            out_d[b, qt * 128 : (qt + 1) * 128, :], o_sb[:]
                        )

    nc.compile()
    return nc


def make_in_maps(inputs, cfg=None):
    c = dict(CFG if cfg is None else cfg)
    B, Q, CV, NCORES, H, CH = c["B"], c["Q"], c["CV"], c["NCORES"], c["H"], c["CH"]
    HD = H * CH
    QL = Q // NCORES
    KB = Q // 128
    CVB = CV // 128
    x = np.ascontiguousarray(np.asarray(inputs["x"], dtype=np.float32))
    b1 = np.asarray(inputs["bias1"], dtype=np.float32)
    b2 = np.asarray(inputs["bias2"], dtype=np.float32)
    wv = np.ascontiguousarray(
        np.asarray(inputs["W_v"], dtype=np.float32).reshape(CVB, 128, HD)
    ).astype(NP_BF16)
    wg = np.ascontiguousarray(
        np.asarray(inputs["W_g"], dtype=np.float32).reshape(CVB, 128, HD)
    ).astype(NP_BF16)
    wo = np.ascontiguousarray(
        np.asarray(inputs["W_o"], dtype=np.float32).reshape(H, CH, CV)
    ).astype(NP_BF16)
    # x^T: [CVB, 128, B*Q] bf16
    xth = np.ascontiguousarray(
        x.reshape(B * Q, CV).T.reshape(CVB, 128, B * Q)
    ).astype(NP_BF16)
    # bias1 per-core: [B,H,128,KB*QL] with b1h[b,h,p,kb*QL+q]=b1[b,h,q0+q,kb*128+p]
    b1g = np.ascontiguousarray(
        b1.reshape(B, H, NCORES, QL, KB, 128).transpose(2, 0, 1, 5, 4, 3)
    ).astype(NP_BF16)
    b2g = np.ascontiguousarray(
        b2.reshape(B, NCORES, QL, KB, 128).transpose(1, 0, 4, 3, 2)
    ).astype(NP_BF16)
    in_maps = []
    for cid in range(NCORES):
        sl = slice(cid * QL, (cid + 1) * QL)
        xlth = np.ascontiguousarray(
            x[:, sl, :].reshape(B * QL, CV).T.reshape(CVB, 128, B * QL)
        ).astype(NP_BF16)
        in_maps.append(
            {
                "b1h": b1g[cid].reshape(B, H, 128, KB * QL),
                "b2h": b2g[cid].reshape(B, 128, KB * QL),
                "xth": xth,
                "xlth": xlth,
                "wvh": wv,
                "wgh": wg,
                "woh": wo,
            }
        )
    return in_maps


_NC_CACHE = {}


def kernel(**inputs) -> np.ndarray:
    key = "main"
    if key not in _NC_CACHE:
        _NC_CACHE[key] = build()
    nc = _NC_CACHE[key]
    in_maps = make_in_maps(inputs)
    res = bass_utils.run_bass_kernel_spmd(nc, in_maps, list(range(CFG["NCORES"])))
    outs = [res.results[cid]["out"] for cid in range(CFG["NCORES"])]
    return np.concatenate(outs, axis=1).astype(np.float32)


# revision 31
# speedup vs baseline: 1.5551x; 1.0182x over previous
"""BiasAttention Trainium2 Bass kernel (v7, bf16 inputs).

Computes, for x:[B,Q,CV], bias1:[B,H,Q,Q], bias2:[B,1,Q,Q], W_v/W_g:[CV,H*CH],
W_o:[H*CH,CV]:

    v = (x @ W_v) viewed [B,Q,H,CH]
    a = softmax(bias1 + bias2, axis=-1)
    o = einsum('bhqk,bhkd->bhqd', a, v) * sigmoid(x @ W_g)
    return o @ W_o

Sharding: data-parallel over the query dim Q across 8 NeuronCores (each core
computes QL=256 query rows end-to-end; no collectives).

Design notes (per core):
  - Inputs are marshalled on the host: per-core slices, transposed so every
    DMA lands 128 partitions with large contiguous per-partition runs, and
    cast to bf16 (the kernel's compute precision; tolerance is 2e-2):
      b1h[b,h,p,kb*QL+q] = bias1[b,h,q0+q,kb*128+p]
      b2h[b,p,kb*QL+q]   = bias2[b,0,q0+q,kb*128+p]
      xth[cb,p,j]        = x[j,cb*128+p]
    This halves HBM traffic (~26MB/core) vs f32 and enables 2x-rate DVE adds.
  - z = b1+b2 in-place on DVE (all-bf16), s = exp(z) on ScalarE (bf16 out),
    attention oT[d,q] accumulates on TensorE per head with a ones-column
    (value 2.0) appended to v for row sums.
  - ScalarE runs ONLY Exp / Tanh / Copy (one ACT table set, no reloads).
    sigmoid(y) = 0.5*(1+tanh(y/2)); the 0.5 folds into the 2.0 ones-column
    (recip = 0.5/sum) and (1+tanh)*recip is one DVE scalar_tensor_tensor.
  - Row-sum reciprocal avoids slow single-partition DVE ops: ACT copies the
    PSUM row to SBUF, gpsimd partition-broadcasts to 64 partitions, DVE runs
    reciprocal_approx_fast wide.
  - Gate is computed transposed (gT = Wg_h^T @ xT) per batch at h==1; the
    per-head epilogue is emitted at the TOP of the iteration two heads later
    so the in-order engine streams never put attention-dependent epilogue
    work ahead of the next head's add/exp.
  - PE warm-up matmuls at t~8us hold the HAM activity window so v-staging
    runs at 2.4GHz; v for batch 1 is prestaged during batch 0's tail heads.
  - The final batch's projection is split: heads 0..6 matmul after epi_6 so
    the kernel tail is only the last head's chain.
"""

import contextlib

import numpy as np


def _ensure_concourse():
    try:
        import concourse  # noqa: F401
    except ImportError:
        import sys

        for p in ("/root/.axon_site/_ro/trn_rl_repo", "/opt/trn_rl_repo"):
            if p not in sys.path:
                sys.path.insert(0, p)


_ensure_concourse()

import ml_dtypes  # noqa: E402
import concourse.bacc as bacc  # noqa: E402
import concourse.mybir as mybir  # noqa: E402
import concourse.tile as tile  # noqa: E402
from concourse import bass_utils  # noqa: E402

F32 = mybir.dt.float32
BF16 = mybir.dt.bfloat16
NP_BF16 = ml_dtypes.bfloat16
AF = mybir.ActivationFunctionType
ALU = mybir.AluOpType

# Problem dims (nn_BiasAttention): hardcoded per the harness contract.
CFG = dict(B=2, Q=2048, CV=512, H=8, CH=64, NCORES=8)


def build(cfg=None, repeat=1, ablate=()):
    c = dict(CFG if cfg is None else cfg)
    B, Q, CV, H, CH, NCORES = c["B"], c["Q"], c["CV"], c["H"], c["CH"], c["NCORES"]
    HD = H * CH
    QL = Q // NCORES  # query rows per core
    KB = Q // 128  # key blocks
    KQ = KB * QL  # free size of one (b,h) bias tile
    CVB = CV // 128
    DH1 = CH + 1  # head dim + ones column (row-sum trick)
    JC = 512  # x^T columns per staged chunk
    KPC = JC // 128  # key blocks covered per x^T chunk
    NCH = B * Q // JC  # number of x^T chunks
    assert QL == 256 and CH == 64 and KQ == 4096

    nc = bacc.Bacc("TRN2", target_bir_lowering=False, debug=False, num_devices=NCORES)

    b1_d = nc.dram_tensor("b1h", [B, H, 128, KQ], BF16, kind="ExternalInput")
    b2_d = nc.dram_tensor("b2h", [B, 128, KQ], BF16, kind="ExternalInput")
    xt_d = nc.dram_tensor("xth", [CVB, 128, B * Q], BF16, kind="ExternalInput")
    xlt_d = nc.dram_tensor("xlth", [CVB, 128, B * QL], BF16, kind="ExternalInput")
    wv_d = nc.dram_tensor("wvh", [CVB, 128, HD], BF16, kind="ExternalInput")
    wg_d = nc.dram_tensor("wgh", [CVB, 128, HD], BF16, kind="ExternalInput")
    wo_d = nc.dram_tensor("woh", [H, CH, CV], BF16, kind="ExternalInput")
    out_d = nc.dram_tensor("out", [B, QL, CV], F32, kind="ExternalOutput")

    with tile.TileContext(nc) as tc:
        loop = tc.For_i(0, repeat, 1) if repeat > 1 else contextlib.nullcontext()
        with loop, contextlib.ExitStack() as ctx:
            persist = ctx.enter_context(tc.tile_pool(name="persist", bufs=1))
            b1p = ctx.enter_context(tc.tile_pool(name="b1p", bufs=5))
            sp = ctx.enter_context(tc.tile_pool(name="sp", bufs=5))
            xbp = ctx.enter_context(tc.tile_pool(name="xbp", bufs=8))
            ep = ctx.enter_context(tc.tile_pool(name="ep", bufs=2))
            op = ctx.enter_context(tc.tile_pool(name="op", bufs=2))
            psA = ctx.enter_context(tc.tile_pool(name="psA", bufs=4, space="PSUM"))
            ps512 = ctx.enter_context(tc.tile_pool(name="ps512", bufs=2, space="PSUM"))
            psG = ctx.enter_context(tc.tile_pool(name="psG", bufs=2, space="PSUM"))

            # ---- persistent tiles ----
            b2_t = persist.tile([128, B * KQ], BF16, name="b2_t", tag="b2_t")
            v_aug = [
                persist.tile([128, KB * H * DH1], BF16, name=f"vaug{b}", tag=f"vaug{b}")
                for b in range(B)
            ]
            xlt = persist.tile([128, CVB * B * QL], BF16, name="xlt", tag="xlt")
            wv_t = persist.tile([128, CVB * HD], BF16, name="wv_t", tag="wv_t")
            wg_t = persist.tile([128, CVB * HD], BF16, name="wg_t", tag="wg_t")
            wo_t = persist.tile([CH, H * CV], BF16, name="wo_t", tag="wo_t")
            gt_t = persist.tile([CH, H * QL], F32, name="gt_t", tag="gt_t")
            og_t = persist.tile([CH, H * QL], BF16, name="og", tag="og")

            # ---- setup ----
            # PE warm-up source: scratch strip of v_aug[0]
            nc.gpsimd.memset(v_aug[0][:, 0 : QL + DH1], 1.0)
            # b2[0] split across both HWDGE queues for startup latency
            nc.sync.dma_start(b2_t[:, 0 : KQ // 2], b2_d[0][:, 0 : KQ // 2])
            nc.scalar.dma_start(
                wv_t[:].rearrange("p (cb d) -> p cb d", cb=CVB),
                wv_d[:, :, :].rearrange("cb p d -> p cb d"),
            )
            nc.scalar.dma_start(
                b2_t[:, KQ // 2 : KQ], b2_d[0][:, KQ // 2 : KQ]
            )
            xt_tiles = {}

            def stage_xt(cidx, eng=None):
                if cidx in xt_tiles:
                    return
                eng = eng or nc.scalar
                xt = xbp.tile([128, CVB * JC], BF16, name=f"xt{cidx}", tag="xt")
                eng.dma_start(
                    xt[:].rearrange("p (cb j) -> p cb j", cb=CVB),
                    xt_d[:, :, cidx * JC : (cidx + 1) * JC].rearrange(
                        "cb p j -> p cb j"
                    ),
                )
                xt_tiles[cidx] = xt

            # b0's x^T chunks + gate weights interleave on the gpsimd queue
            stage_xt(0, nc.gpsimd)
            nc.gpsimd.dma_start(
                wg_t[:].rearrange("p (cb d) -> p cb d", cb=CVB),
                wg_d[:, :, :].rearrange("cb p d -> p cb d"),
            )
            stage_xt(1, nc.gpsimd)
            nc.gpsimd.dma_start(
                xlt[:].rearrange("p (cb j) -> p cb j", cb=CVB),
                xlt_d[:, :, :].rearrange("cb p j -> p cb j"),
            )
            stage_xt(2, nc.gpsimd)
            stage_xt(3, nc.gpsimd)
            # warm-up matmuls (results discarded; psA slot recycled later)
            warm_ps = psA.tile([DH1, 2 * QL], F32, name="warm", tag="oT")
            for i in range(36):
                nc.tensor.matmul(
                    warm_ps[:, 0:QL],
                    v_aug[0][:, 0:DH1],
                    v_aug[0][:, DH1 : DH1 + QL],
                    start=(i == 0),
                    stop=(i == 35),
                )
            # ones columns (value 2.0: folds the 0.5 of the tanh-sigmoid)
            for b in range(B):
                ones_ap = v_aug[b][:].rearrange("p (n d) -> p n d", d=DH1)[:, :, CH]
                nc.gpsimd.memset(ones_ap, 2.0)

            def stage_v(b, kb):
                """v row-tile for keys [kb*128,(kb+1)*128) of batch b."""
                if "stagev" in ablate:
                    return
                cidx = (b * Q + kb * 128) // JC
                loc = (kb % KPC) * 128
                stage_xt(cidx)
                xtb = xt_tiles[cidx]
                v_ps = ps512.tile([128, HD], F32, name="v_ps", tag="ps512")
                for cb in range(CVB):
                    nc.tensor.matmul(
                        v_ps[:],
                        xtb[:, cb * JC + loc : cb * JC + loc + 128],
                        wv_t[:, cb * HD : (cb + 1) * HD],
                        start=(cb == 0),
                        stop=(cb == CVB - 1),
                    )
                dst = v_aug[b][:].rearrange("p (kt h d) -> p kt h d", h=H, d=DH1)[
                    :, kb, :, 0:CH
                ]
                src = v_ps[:].rearrange("p (h d) -> p h d", d=CH)
                nc.vector.tensor_copy(dst, src)

            def emit_epi(b, h, oT):
                """og_h = oT * (1+tanh)*recip  (bf16), recip = 0.5/rowsum."""
                if "epi" in ablate:
                    return
                rs = ep.tile([1, QL], F32, name="rs", tag="rs")
                nc.scalar.copy(rs[:], oT[CH : CH + 1, 0:QL])
                rsB = ep.tile([CH, QL], F32, name="rsB", tag="rsB")
                nc.gpsimd.partition_broadcast(rsB[:], rs[0:1, :])
                rT = ep.tile([CH, QL], F32, name="rT", tag="rT")
                nc.vector.reciprocal_approx_fast(rT[:], rsB[:])
                t1 = ep.tile([CH, QL], F32, name="t1", tag="t1")
                nc.vector.scalar_tensor_tensor(
                    t1[:], gt_t[:, h * QL : (h + 1) * QL], 1.0, rT[:],
                    ALU.add, ALU.mult,
                )
                nc.vector.tensor_mul(
                    og_t[:, h * QL : (h + 1) * QL], t1[:], oT[0:CH, 0:QL]
                )

            def emit_gates(b):
                """gates for all heads of b (transposed): tanh(0.5*Wg_h^T xT)"""
                if "gate" in ablate:
                    return
                for h in range(H):
                    gt_ps = psG.tile([CH, QL], F32, name="gt_ps", tag="gt_ps")
                    for cb in range(CVB):
                        nc.tensor.matmul(
                            gt_ps[:],
                            wg_t[:, cb * HD + h * CH : cb * HD + (h + 1) * CH],
                            xlt[:, cb * B * QL + b * QL : cb * B * QL + (b + 1) * QL],
                            start=(cb == 0),
                            stop=(cb == CVB - 1),
                        )
                    nc.scalar.activation(
                        gt_t[:, h * QL : (h + 1) * QL], gt_ps[:], AF.Tanh,
                        scale=0.5,
                    )

            def produce(b, h, halves=1):
                """b1 DMA + z-add + exp for head (b,h); returns sT."""
                b1t = b1p.tile([128, KQ], BF16, name="b1t", tag="b1t")
                sT = sp.tile([128, KQ], BF16, name="sT", tag="sT")
                K2 = KQ // 2
                for chk in range(2):
                    if "b1dma" not in ablate:
                        eng = nc.scalar if (halves == 2 and chk == 1) else nc.sync
                        eng.dma_start(
                            b1t[:, chk * K2 : (chk + 1) * K2],
                            b1_d[b, h, :, chk * K2 : (chk + 1) * K2],
                        )
                    if halves == 2:
                        if "zadd" not in ablate:
                            nc.vector.tensor_add(
                                b1t[:, chk * K2 : (chk + 1) * K2],
                                b1t[:, chk * K2 : (chk + 1) * K2],
                                b2_t[:, b * KQ + chk * K2 : b * KQ + (chk + 1) * K2],
                            )
                        if "exp" not in ablate:
                            nc.scalar.activation(
                                sT[:, chk * K2 : (chk + 1) * K2],
                                b1t[:, chk * K2 : (chk + 1) * K2],
                                AF.Exp,
                            )
                if halves == 1:
                    if "zadd" not in ablate:
                        nc.vector.tensor_add(
                            b1t[:], b1t[:], b2_t[:, b * KQ : (b + 1) * KQ]
                        )
                    if "exp" not in ablate:
                        nc.scalar.activation(sT[:], b1t[:], AF.Exp)
                return sT

            # ---- main per-batch flow ----
            prestaged = [0]
            for b in range(B):
                pend = []
                sT_next = produce(b, 0, halves=2 if b == 0 else 1)
                for h in range(H):
                    # deferred epilogue first: its ACT/gpsimd/DVE ops must
                    # precede the next exp in each engine's in-order stream
                    if len(pend) >= 2:
                        ph, poT = pend.pop(0)
                        emit_epi(b, ph, poT)
                    if h == 0:
                        emit_gates(b)
                    if b == 0 and h == 1:
                        for ci in range(NCH // 2, NCH):
                            stage_xt(ci)
                    if b == 0 and h == 2:
                        nc.gpsimd.dma_start(
                            wo_t[:].rearrange("p (hh cc) -> p hh cc", hh=H),
                            wo_d[:, :, :].rearrange("h p c -> p h c"),
                        )
                    if b == 0 and h == 5:
                        nc.scalar.dma_start(b2_t[:, KQ : 2 * KQ], b2_d[1])

                    sT = sT_next
                    if h + 1 < H:
                        sT_next = produce(b, h + 1)

                    # attention: oT[d,q] (+2*rowsum via ones col) on TensorE
                    oT = psA.tile([DH1, 2 * QL], F32, name="oT", tag="oT")
                    if "attn" not in ablate:
                        for kb in range(KB):
                            if h == 0 and not (b == 1 and kb < prestaged[0]):
                                stage_v(b, kb)
                            # prestage next batch's v during b0's tail heads
                            if b == 0 and h >= 5 and kb % 3 == 0 and prestaged[0] < KB:
                                stage_v(1, prestaged[0])
                                prestaged[0] += 1
                            base = (kb * H + h) * DH1
                            nc.tensor.matmul(
                                oT[:, 0:QL],
                                v_aug[b][:, base : base + DH1],
                                sT[:, kb * QL : (kb + 1) * QL],
                                start=(kb == 0),
                                stop=(kb == KB - 1),
                            )

                    pend.append((h, oT))

                if "epi" in ablate:
                    continue
                if b != B - 1:
                    for ph, poT in pend:
                        emit_epi(b, ph, poT)
                    for qt in range(QL // 128):
                        o_ps = ps512.tile([128, CV], F32, name="o_ps", tag="ps512")
                        for hh in range(H):
                            nc.tensor.matmul(
                                o_ps[:],
                                og_t[:, hh * QL + qt * 128 : hh * QL + qt * 128 + 128],
                                wo_t[:, hh * CV : (hh + 1) * CV],
                                start=(hh == 0),
                                stop=(hh == H - 1),
                            )
                        o_sb = op.tile([128, CV], F32, name="o_sb", tag="o_sb")
                        nc.vector.tensor_copy(o_sb[:], o_ps[:])
                        nc.scalar.dma_start(
                            out_d[b, qt * 128 : (qt + 1) * 128, :], o_sb[:]
                        )
                else:
                    # split tail: epi_6, project heads 0..6, epi_7, finish
                    o_pss = [
                        ps512.tile([128, CV], F32, name=f"o_ps{qt}", tag="ps512")
                        for qt in range(QL // 128)
                    ]

                    def proj(hh, qt):
                        nc.tensor.matmul(
                            o_pss[qt][:],
                            og_t[:, hh * QL + qt * 128 : hh * QL + qt * 128 + 128],
                            wo_t[:, hh * CV : (hh + 1) * CV],
                            start=(hh == 0),
                            stop=(hh == H - 1),
                        )

                    emit_epi(b, pend[0][0], pend[0][1])
                    for qt in range(QL // 128):
                        for hh in range(H - 1):
                            proj(hh, qt)
                    emit_epi(b, pend[1][0], pend[1][1])
                    for qt in range(QL // 128):
                        proj(H - 1, qt)
                        o_sb = op.tile([128, CV], F32, name="o_sb", tag="o_sb")
                        nc.vector.tensor_copy(o_sb[:], o_pss[qt][:])
                        nc.scalar.dma_start(
                            out_d[b, qt * 128 : (qt + 1) * 128, :], o_sb[:]
                        )

    nc.compile()
    return nc


def make_in_maps(inputs, cfg=None):
    c = dict(CFG if cfg is None else cfg)
    B, Q, CV, NCORES, H, CH = c["B"], c["Q"], c["CV"], c["NCORES"], c["H"], c["CH"]
    HD = H * CH
    QL = Q // NCORES
    KB = Q // 128
    CVB = CV // 128
    x = np.ascontiguousarray(np.asarray(inputs["x"], dtype=np.float32))
    b1 = np.asarray(inputs["bias1"], dtype=np.float32)
    b2 = np.asarray(inputs["bias2"], dtype=np.float32)
    wv = np.ascontiguousarray(
        np.asarray(inputs["W_v"], dtype=np.float32).reshape(CVB, 128, HD)
    ).astype(NP_BF16)
    wg = np.ascontiguousarray(
        np.asarray(inputs["W_g"], dtype=np.float32).reshape(CVB, 128, HD)
    ).astype(NP_BF16)
    wo = np.ascontiguousarray(
        np.asarray(inputs["W_o"], dtype=np.float32).reshape(H, CH, CV)
    ).astype(NP_BF16)
    # x^T: [CVB, 128, B*Q] bf16
    xth = np.ascontiguousarray(
        x.reshape(B * Q, CV).T.reshape(CVB, 128, B * Q)
    ).astype(NP_BF16)
    # bias1 per-core: [B,H,128,KB*QL] with b1h[b,h,p,kb*QL+q]=b1[b,h,q0+q,kb*128+p]
    b1g = np.ascontiguousarray(
        b1.reshape(B, H, NCORES, QL, KB, 128).transpose(2, 0, 1, 5, 4, 3)
    ).astype(NP_BF16)
    b2g = np.ascontiguousarray(
        b2.reshape(B, NCORES, QL, KB, 128).transpose(1, 0, 4, 3, 2)
    ).astype(NP_BF16)
    in_maps = []
    for cid in range(NCORES):
        sl = slice(cid * QL, (cid + 1) * QL)
        xlth = np.ascontiguousarray(
            x[:, sl, :].reshape(B * QL, CV).T.reshape(CVB, 128, B * QL)
        ).astype(NP_BF16)
        in_maps.append(
            {
                "b1h": b1g[cid].reshape(B, H, 128, KB * QL),
                "b2h": b2g[cid].reshape(B, 128, KB * QL),
                "xth": xth,
                "xlth": xlth,
                "wvh": wv,
                "wgh": wg,
                "woh": wo,
            }
        )
    return in_maps


_NC_CACHE = {}


def kernel(**inputs) -> np.ndarray:
    key = "main"
    if key not in _NC_CACHE:
        _NC_CACHE[key] = build()
    nc = _NC_CACHE[key]
    in_maps = make_in_maps(inputs)
    res = bass_utils.run_bass_kernel_spmd(nc, in_maps, list(range(CFG["NCORES"])))
    outs = [res.results[cid]["out"] for cid in range(CFG["NCORES"])]
    return np.concatenate(outs, axis=1).astype(np.float32)


# revision 32
# speedup vs baseline: 1.6160x; 1.0392x over previous
"""BiasAttention Trainium2 Bass kernel (v7, bf16 inputs).

Computes, for x:[B,Q,CV], bias1:[B,H,Q,Q], bias2:[B,1,Q,Q], W_v/W_g:[CV,H*CH],
W_o:[H*CH,CV]:

    v = (x @ W_v) viewed [B,Q,H,CH]
    a = softmax(bias1 + bias2, axis=-1)
    o = einsum('bhqk,bhkd->bhqd', a, v) * sigmoid(x @ W_g)
    return o @ W_o

Sharding: data-parallel over the query dim Q across 8 NeuronCores (each core
computes QL=256 query rows end-to-end; no collectives).

Design notes (per core):
  - Inputs are marshalled on the host: per-core slices, transposed so every
    DMA lands 128 partitions with large contiguous per-partition runs, and
    cast to bf16 (the kernel's compute precision; tolerance is 2e-2):
      b1h[b,h,p,kb*QL+q] = bias1[b,h,q0+q,kb*128+p]
      b2h[b,p,kb*QL+q]   = bias2[b,0,q0+q,kb*128+p]
      xth[cb,p,j]        = x[j,cb*128+p]
    This halves HBM traffic (~26MB/core) vs f32 and enables 2x-rate DVE adds.
  - z = b1+b2 in-place on DVE (all-bf16), s = exp(z) on ScalarE (bf16 out),
    attention oT[d,q] accumulates on TensorE per head with a ones-column
    (value 2.0) appended to v for row sums.
  - ScalarE runs ONLY Exp / Tanh / Copy (one ACT table set, no reloads).
    sigmoid(y) = 0.5*(1+tanh(y/2)); the 0.5 folds into the 2.0 ones-column
    (recip = 0.5/sum) and (1+tanh)*recip is one DVE scalar_tensor_tensor.
  - Row-sum reciprocal avoids slow single-partition DVE ops: ACT copies the
    PSUM row to SBUF, gpsimd partition-broadcasts to 64 partitions, DVE runs
    reciprocal_approx_fast wide.
  - Gate is computed transposed (gT = Wg_h^T @ xT) per batch at h==1; the
    per-head epilogue is emitted at the TOP of the iteration two heads later
    so the in-order engine streams never put attention-dependent epilogue
    work ahead of the next head's add/exp.
  - PE warm-up matmuls at t~8us hold the HAM activity window so v-staging
    runs at 2.4GHz; v for batch 1 is prestaged during batch 0's tail heads.
  - The final batch's projection is split: heads 0..6 matmul after epi_6 so
    the kernel tail is only the last head's chain.
"""

import contextlib

import numpy as np


def _ensure_concourse():
    try:
        import concourse  # noqa: F401
    except ImportError:
        import sys

        for p in ("/root/.axon_site/_ro/trn_rl_repo", "/opt/trn_rl_repo"):
            if p not in sys.path:
                sys.path.insert(0, p)


_ensure_concourse()

import ml_dtypes  # noqa: E402
import concourse.bacc as bacc  # noqa: E402
import concourse.mybir as mybir  # noqa: E402
import concourse.tile as tile  # noqa: E402
from concourse import bass_utils  # noqa: E402

F32 = mybir.dt.float32
BF16 = mybir.dt.bfloat16
NP_BF16 = ml_dtypes.bfloat16
AF = mybir.ActivationFunctionType
ALU = mybir.AluOpType

# Problem dims (nn_BiasAttention): hardcoded per the harness contract.
CFG = dict(B=2, Q=2048, CV=512, H=8, CH=64, NCORES=8)


def build(cfg=None, repeat=1, ablate=()):
    c = dict(CFG if cfg is None else cfg)
    B, Q, CV, H, CH, NCORES = c["B"], c["Q"], c["CV"], c["H"], c["CH"], c["NCORES"]
    HD = H * CH
    QL = Q // NCORES  # query rows per core
    KB = Q // 128  # key blocks
    KQ = KB * QL  # free size of one (b,h) bias tile
    CVB = CV // 128
    DH1 = CH + 1  # head dim + ones column (row-sum trick)
    JC = 512  # x^T columns per staged chunk
    KPC = JC // 128  # key blocks covered per x^T chunk
    NCH = B * Q // JC  # number of x^T chunks
    assert QL == 256 and CH == 64 and KQ == 4096

    nc = bacc.Bacc("TRN2", target_bir_lowering=False, debug=False, num_devices=NCORES)

    b1_d = nc.dram_tensor("b1h", [B, H, 128, KQ], BF16, kind="ExternalInput")
    b2_d = nc.dram_tensor("b2h", [B, 128, KQ], BF16, kind="ExternalInput")
    xt_d = nc.dram_tensor("xth", [CVB, 128, B * Q], BF16, kind="ExternalInput")
    xlt_d = nc.dram_tensor("xlth", [CVB, 128, B * QL], BF16, kind="ExternalInput")
    wv_d = nc.dram_tensor("wvh", [CVB, 128, HD], BF16, kind="ExternalInput")
    wg_d = nc.dram_tensor("wgh", [CVB, 128, HD], BF16, kind="ExternalInput")
    wo_d = nc.dram_tensor("woh", [H, CH, CV], BF16, kind="ExternalInput")
    out_d = nc.dram_tensor("out", [B, QL, CV], F32, kind="ExternalOutput")

    with tile.TileContext(nc) as tc:
        loop = tc.For_i(0, repeat, 1) if repeat > 1 else contextlib.nullcontext()
        with loop, contextlib.ExitStack() as ctx:
            persist = ctx.enter_context(tc.tile_pool(name="persist", bufs=1))
            b1p = ctx.enter_context(tc.tile_pool(name="b1p", bufs=5))
            sp = ctx.enter_context(tc.tile_pool(name="sp", bufs=5))
            xbp = ctx.enter_context(tc.tile_pool(name="xbp", bufs=8))
            ep = ctx.enter_context(tc.tile_pool(name="ep", bufs=2))
            op = ctx.enter_context(tc.tile_pool(name="op", bufs=2))
            psA = ctx.enter_context(tc.tile_pool(name="psA", bufs=4, space="PSUM"))
            ps512 = ctx.enter_context(tc.tile_pool(name="ps512", bufs=2, space="PSUM"))
            psG = ctx.enter_context(tc.tile_pool(name="psG", bufs=2, space="PSUM"))

            # ---- persistent tiles ----
            b2_t = persist.tile([128, B * KQ], BF16, name="b2_t", tag="b2_t")
            v_aug = [
                persist.tile([128, KB * H * DH1], BF16, name=f"vaug{b}", tag=f"vaug{b}")
                for b in range(B)
            ]
            xlt = persist.tile([128, CVB * B * QL], BF16, name="xlt", tag="xlt")
            wv_t = persist.tile([128, CVB * HD], BF16, name="wv_t", tag="wv_t")
            wg_t = persist.tile([128, CVB * HD], BF16, name="wg_t", tag="wg_t")
            wo_t = persist.tile([CH, H * CV], BF16, name="wo_t", tag="wo_t")
            gt_t = persist.tile([CH, H * QL], F32, name="gt_t", tag="gt_t")
            og_t = persist.tile([CH, H * QL], BF16, name="og", tag="og")

            # ---- setup ----
            # PE warm-up source: scratch strip of v_aug[0]
            nc.gpsimd.memset(v_aug[0][:, 0 : QL + DH1], 1.0)
            # b2[0] split across both HWDGE queues for startup latency
            nc.sync.dma_start(b2_t[:, 0 : KQ // 2], b2_d[0][:, 0 : KQ // 2])
            nc.scalar.dma_start(
                wv_t[:].rearrange("p (cb d) -> p cb d", cb=CVB),
                wv_d[:, :, :].rearrange("cb p d -> p cb d"),
            )
            nc.scalar.dma_start(
                b2_t[:, KQ // 2 : KQ], b2_d[0][:, KQ // 2 : KQ]
            )
            xt_tiles = {}

            def stage_xt(cidx, eng=None):
                if cidx in xt_tiles:
                    return
                eng = eng or nc.scalar
                xt = xbp.tile([128, CVB * JC], BF16, name=f"xt{cidx}", tag="xt")
                eng.dma_start(
                    xt[:].rearrange("p (cb j) -> p cb j", cb=CVB),
                    xt_d[:, :, cidx * JC : (cidx + 1) * JC].rearrange(
                        "cb p j -> p cb j"
                    ),
                )
                xt_tiles[cidx] = xt

            # b0's x^T chunks + gate weights interleave on the gpsimd queue
            stage_xt(0, nc.gpsimd)
            nc.gpsimd.dma_start(
                wg_t[:].rearrange("p (cb d) -> p cb d", cb=CVB),
                wg_d[:, :, :].rearrange("cb p d -> p cb d"),
            )
            stage_xt(1, nc.gpsimd)
            nc.gpsimd.dma_start(
                xlt[:].rearrange("p (cb j) -> p cb j", cb=CVB),
                xlt_d[:, :, :].rearrange("cb p j -> p cb j"),
            )
            stage_xt(2, nc.gpsimd)
            stage_xt(3, nc.gpsimd)
            # warm-up matmuls (results discarded; psA slot recycled later)
            warm_ps = psA.tile([DH1, 2 * QL], F32, name="warm", tag="oT")
            for i in range(36):
                nc.tensor.matmul(
                    warm_ps[:, 0:QL],
                    v_aug[0][:, 0:DH1],
                    v_aug[0][:, DH1 : DH1 + QL],
                    start=(i == 0),
                    stop=(i == 35),
                )
            # ones columns (value 2.0: folds the 0.5 of the tanh-sigmoid)
            for b in range(B):
                ones_ap = v_aug[b][:].rearrange("p (n d) -> p n d", d=DH1)[:, :, CH]
                nc.gpsimd.memset(ones_ap, 2.0)

            def stage_v(b, kb):
                """v row-tile for keys [kb*128,(kb+1)*128) of batch b."""
                if "stagev" in ablate:
                    return
                cidx = (b * Q + kb * 128) // JC
                loc = (kb % KPC) * 128
                stage_xt(cidx)
                xtb = xt_tiles[cidx]
                v_ps = ps512.tile([128, HD], F32, name="v_ps", tag="ps512")
                for cb in range(CVB):
                    nc.tensor.matmul(
                        v_ps[:],
                        xtb[:, cb * JC + loc : cb * JC + loc + 128],
                        wv_t[:, cb * HD : (cb + 1) * HD],
                        start=(cb == 0),
                        stop=(cb == CVB - 1),
                    )
                dst = v_aug[b][:].rearrange("p (kt h d) -> p kt h d", h=H, d=DH1)[
                    :, kb, :, 0:CH
                ]
                src = v_ps[:].rearrange("p (h d) -> p h d", d=CH)
                nc.vector.tensor_copy(dst, src)

            def emit_epi(b, h, oT):
                """og_h = oT * (1+tanh)*recip  (bf16), recip = 0.5/rowsum."""
                if "epi" in ablate:
                    return
                rs = ep.tile([1, QL], F32, name="rs", tag="rs")
                nc.scalar.copy(rs[:], oT[CH : CH + 1, 0:QL])
                rsB = ep.tile([CH, QL], F32, name="rsB", tag="rsB")
                nc.gpsimd.partition_broadcast(rsB[:], rs[0:1, :])
                rT = ep.tile([CH, QL], F32, name="rT", tag="rT")
                nc.vector.reciprocal_approx_fast(rT[:], rsB[:])
                t1 = ep.tile([CH, QL], F32, name="t1", tag="t1")
                nc.vector.scalar_tensor_tensor(
                    t1[:], gt_t[:, h * QL : (h + 1) * QL], 1.0, rT[:],
                    ALU.add, ALU.mult,
                )
                nc.vector.tensor_mul(
                    og_t[:, h * QL : (h + 1) * QL], t1[:], oT[0:CH, 0:QL]
                )

            def emit_gates(b):
                """gates for all heads of b (transposed): tanh(0.5*Wg_h^T xT)"""
                if "gate" in ablate:
                    return
                for h in range(H):
                    gt_ps = psG.tile([CH, QL], F32, name="gt_ps", tag="gt_ps")
                    for cb in range(CVB):
                        nc.tensor.matmul(
                            gt_ps[:],
                            wg_t[:, cb * HD + h * CH : cb * HD + (h + 1) * CH],
                            xlt[:, cb * B * QL + b * QL : cb * B * QL + (b + 1) * QL],
                            start=(cb == 0),
                            stop=(cb == CVB - 1),
                        )
                    nc.scalar.activation(
                        gt_t[:, h * QL : (h + 1) * QL], gt_ps[:], AF.Tanh,
                        scale=0.5,
                    )

            def produce(b, h, halves=1):
                """b1 DMA + z-add + exp for head (b,h); returns sT."""
                b1t = b1p.tile([128, KQ], BF16, name="b1t", tag="b1t")
                sT = sp.tile([128, KQ], BF16, name="sT", tag="sT")
                K2 = KQ // 2
                for chk in range(2):
                    if "b1dma" not in ablate:
                        eng = nc.scalar if (halves == 2 and chk == 1) else nc.sync
                        eng.dma_start(
                            b1t[:, chk * K2 : (chk + 1) * K2],
                            b1_d[b, h, :, chk * K2 : (chk + 1) * K2],
                        )
                    if halves == 2:
                        if "zadd" not in ablate:
                            nc.vector.tensor_add(
                                b1t[:, chk * K2 : (chk + 1) * K2],
                                b1t[:, chk * K2 : (chk + 1) * K2],
                                b2_t[:, b * KQ + chk * K2 : b * KQ + (chk + 1) * K2],
                            )
                        if "exp" not in ablate:
                            nc.scalar.activation(
                                sT[:, chk * K2 : (chk + 1) * K2],
                                b1t[:, chk * K2 : (chk + 1) * K2],
                                AF.Exp,
                            )
                if halves == 1:
                    if "zadd" not in ablate:
                        nc.vector.tensor_add(
                            b1t[:], b1t[:], b2_t[:, b * KQ : (b + 1) * KQ]
                        )
                    if "exp" not in ablate:
                        nc.scalar.activation(sT[:], b1t[:], AF.Exp)
                return sT

            # ---- main per-batch flow ----
            prestaged = [0]
            for b in range(B):
                pend = []
                sTq = [produce(b, 0, halves=2 if b == 0 else 1)]
                sTq.append(produce(b, 1))
                sTq.append(produce(b, 2))
                for h in range(H):
                    # deferred epilogue first: its ACT/gpsimd/DVE ops must
                    # precede the next exp in each engine's in-order stream
                    if len(pend) >= 2:
                        ph, poT = pend.pop(0)
                        emit_epi(b, ph, poT)
                    if h == 0:
                        emit_gates(b)
                    if b == 0 and h == 1:
                        for ci in range(NCH // 2, NCH):
                            stage_xt(ci)
                    if b == 0 and h == 2:
                        nc.gpsimd.dma_start(
                            wo_t[:].rearrange("p (hh cc) -> p hh cc", hh=H),
                            wo_d[:, :, :].rearrange("h p c -> p h c"),
                        )
                    if b == 0 and h == 5:
                        nc.scalar.dma_start(b2_t[:, KQ : 2 * KQ], b2_d[1])

                    sT = sTq.pop(0)
                    if h + 3 < H:
                        sTq.append(produce(b, h + 3))

                    # attention: oT[d,q] (+2*rowsum via ones col) on TensorE
                    oT = psA.tile([DH1, 2 * QL], F32, name="oT", tag="oT")
                    if "attn" not in ablate:
                        for kb in range(KB):
                            if h == 0 and not (b == 1 and kb < prestaged[0]):
                                stage_v(b, kb)
                            # prestage next batch's v during b0's tail heads
                            if b == 0 and h >= 5 and kb % 3 == 0 and prestaged[0] < KB:
                                stage_v(1, prestaged[0])
                                prestaged[0] += 1
                            base = (kb * H + h) * DH1
                            nc.tensor.matmul(
                                oT[:, 0:QL],
                                v_aug[b][:, base : base + DH1],
                                sT[:, kb * QL : (kb + 1) * QL],
                                start=(kb == 0),
                                stop=(kb == KB - 1),
                            )

                    pend.append((h, oT))

                if "epi" in ablate:
                    continue
                if b != B - 1:
                    for ph, poT in pend:
                        emit_epi(b, ph, poT)
                    for qt in range(QL // 128):
                        o_ps = ps512.tile([128, CV], F32, name="o_ps", tag="ps512")
                        for hh in range(H):
                            nc.tensor.matmul(
                                o_ps[:],
                                og_t[:, hh * QL + qt * 128 : hh * QL + qt * 128 + 128],
                                wo_t[:, hh * CV : (hh + 1) * CV],
                                start=(hh == 0),
                                stop=(hh == H - 1),
                            )
                        o_sb = op.tile([128, CV], F32, name="o_sb", tag="o_sb")
                        nc.vector.tensor_copy(o_sb[:], o_ps[:])
                        nc.scalar.dma_start(
                            out_d[b, qt * 128 : (qt + 1) * 128, :], o_sb[:]
                        )
                else:
                    # split tail: epi_6, project heads 0..6, epi_7, finish
                    o_pss = [
                        ps512.tile([128, CV], F32, name=f"o_ps{qt}", tag="ps512")
                        for qt in range(QL // 128)
                    ]

                    def proj(hh, qt):
                        nc.tensor.matmul(
                            o_pss[qt][:],
                            og_t[:, hh * QL + qt * 128 : hh * QL + qt * 128 + 128],
                            wo_t[:, hh * CV : (hh + 1) * CV],
                            start=(hh == 0),
                            stop=(hh == H - 1),
                        )

                    emit_epi(b, pend[0][0], pend[0][1])
                    for qt in range(QL // 128):
                        for hh in range(H - 1):
                            proj(hh, qt)
                    emit_epi(b, pend[1][0], pend[1][1])
                    for qt in range(QL // 128):
                        proj(H - 1, qt)
                        o_sb = op.tile([128, CV], F32, name="o_sb", tag="o_sb")
                        nc.vector.tensor_copy(o_sb[:], o_pss[qt][:])
                        nc.scalar.dma_start(
                            out_d[b, qt * 128 : (qt + 1) * 128, :], o_sb[:]
                        )

    nc.compile()
    return nc


def make_in_maps(inputs, cfg=None):
    c = dict(CFG if cfg is None else cfg)
    B, Q, CV, NCORES, H, CH = c["B"], c["Q"], c["CV"], c["NCORES"], c["H"], c["CH"]
    HD = H * CH
    QL = Q // NCORES
    KB = Q // 128
    CVB = CV // 128
    x = np.ascontiguousarray(np.asarray(inputs["x"], dtype=np.float32))
    b1 = np.asarray(inputs["bias1"], dtype=np.float32)
    b2 = np.asarray(inputs["bias2"], dtype=np.float32)
    wv = np.ascontiguousarray(
        np.asarray(inputs["W_v"], dtype=np.float32).reshape(CVB, 128, HD)
    ).astype(NP_BF16)
    wg = np.ascontiguousarray(
        np.asarray(inputs["W_g"], dtype=np.float32).reshape(CVB, 128, HD)
    ).astype(NP_BF16)
    wo = np.ascontiguousarray(
        np.asarray(inputs["W_o"], dtype=np.float32).reshape(H, CH, CV)
    ).astype(NP_BF16)
    # x^T: [CVB, 128, B*Q] bf16
    xth = np.ascontiguousarray(
        x.reshape(B * Q, CV).T.reshape(CVB, 128, B * Q)
    ).astype(NP_BF16)
    # bias1 per-core: [B,H,128,KB*QL] with b1h[b,h,p,kb*QL+q]=b1[b,h,q0+q,kb*128+p]
    b1g = np.ascontiguousarray(
        b1.reshape(B, H, NCORES, QL, KB, 128).transpose(2, 0, 1, 5, 4, 3)
    ).astype(NP_BF16)
    b2g = np.ascontiguousarray(
        b2.reshape(B, NCORES, QL, KB, 128).transpose(1, 0, 4, 3, 2)
    ).astype(NP_BF16)
    in_maps = []
    for cid in range(NCORES):
        sl = slice(cid * QL, (cid + 1) * QL)
        xlth = np.ascontiguousarray(
            x[:, sl, :].reshape(B * QL, CV).T.reshape(CVB, 128, B * QL)
        ).astype(NP_BF16)
        in_maps.append(
            {
                "b1h": b1g[cid].reshape(B, H, 128, KB * QL),
                "b2h": b2g[cid].reshape(B, 128, KB * QL),
                "xth": xth,
                "xlth": xlth,
                "wvh": wv,
                "wgh": wg,
                "woh": wo,
            }
        )
    return in_maps


_NC_CACHE = {}


def kernel(**inputs) -> np.ndarray:
    key = "main"
    if key not in _NC_CACHE:
        _NC_CACHE[key] = build()
    nc = _NC_CACHE[key]
    in_maps = make_in_maps(inputs)
    res = bass_utils.run_bass_kernel_spmd(nc, in_maps, list(range(CFG["NCORES"])))
    outs = [res.results[cid]["out"] for cid in range(CFG["NCORES"])]
    return np.concatenate(outs, axis=1).astype(np.float32)
